# revision 1
# baseline (speedup 1.0000x reference)
"""Trainium2 Bass kernel for nn_AttentionWeight (GAT edge softmax).

out[e,h] = softmax_over_dst_segments(relu(el[src]+er[dst]+ee[etype]))

Math used on device:
  exp(relu(x)) = max(exp(x), 1)  and  exp(x) = exp(el+ee)*exp(er)
  y := exp(relu(x)) - 1 = max(exp(el+ee)*exp(er) - 1, 0)
  segment_sum(exp(relu(x))) = sum(y) + deg   (padding slots give y = 0)
  out = (y + 1) * reciprocal(segment_sum)    (softmax is shift-invariant, the
                                              reference's max-subtraction is
                                              only for numerical range; values
                                              here are O(1) so it is not needed)

Distribution (8 NeuronCores):
  Launch A: node-sharded projections. Core s owns nodes [12500s, 12500(s+1)):
    el/er = feat @ (W_fc contracted with attn_l/attn_r) -> exp'd; the tiny
    edge-type table ee' = exp(contract(edge_emb@W_e, attn_e)); and the
    combined gather table el8[(n,t)] = el'[n]*ee'[t] for its node shard.
  Host: concatenates per-core el8 shards (pure relabeling, no arithmetic).
  Launch B: edge/dst-sharded softmax. Core c owns dst in [12500c, 12500(c+1)).
    Edges are dst-sorted and padded into [128 nodes x D_g] groups (nodes
    degree-sorted so groups are tight, ~1.5%% padding). One [128,1]-indexed
    indirect DMA gathers one slot column (128 rows of 32B) from el8; walrus
    miscompiles multi-index offset APs, so one instruction per column is the
    only correct form, and its ~1us SWDGE fixed cost on the Pool engine is
    the kernel's dominant term. Per group: multiply by broadcast er', the
    max(m-1,0) trick, a strided X-reduce for segment sums, reciprocal, and
    (y+1)*r, then store the padded slots.
  Host: scatters padded slots back to original edge order (indexing only).

All floating-point arithmetic happens on device; the host only shards,
permutes, concatenates and builds integer index/count arrays.
"""

import sys

sys.path.insert(0, "/opt/trn_rl_repo")

import numpy as np

import concourse.bass as bass
import concourse.bacc as bacc
import concourse.mybir as mybir
import concourse.tile as tile
from concourse.bass_utils import run_bass_kernel_spmd

# problem constants (hardcoded per harness contract)
N = 100000
E = 3200000
IN = 256
H = 8
O = 64
F = 64
T = 8
NCORES = 8
P = 128

NS = N // NCORES            # 12500 nodes per shard
NSP = 12544                 # padded to 128*98
G = NSP // P                # 98 groups of 128 nodes
ELFULL_ROWS = 128 * 785     # 100480: 8*12544=100352 real rows + pad
SENTINEL = 100352           # zero row in el_full -> el8 row SENTINEL*8 is 0
EL8_ROWS = ELFULL_ROWS * 8

FP = mybir.dt.float32
I32 = mybir.dt.int32

_timings = {}


# ---------------------------------------------------------------------------
# Launch A: projections
# ---------------------------------------------------------------------------

def _build_launch_a():
    nc = bacc.Bacc("TRN2", target_bir_lowering=False, debug=False,
                   num_devices=NCORES)
    featT = nc.dram_tensor("featT", [IN, NSP], FP, kind="ExternalInput")
    w_fc = nc.dram_tensor("w_fc", [IN, H * O], FP, kind="ExternalInput")
    attn_lr = nc.dram_tensor("attn_lr", [P, 2 * H * O], FP, kind="ExternalInput")
    edge_embT = nc.dram_tensor("edge_embT", [F, T], FP, kind="ExternalInput")
    w_e = nc.dram_tensor("w_e", [F, H * F], FP, kind="ExternalInput")
    attn_e = nc.dram_tensor("attn_e", [T, H * F], FP, kind="ExternalInput")
    erp = nc.dram_tensor("erp", [NSP, H], FP, kind="ExternalOutput")
    eep = nc.dram_tensor("eep", [T, H], FP, kind="ExternalOutput")
    el8s = nc.dram_tensor("el8s", [NSP * T, H], FP, kind="ExternalOutput")

    with tile.TileContext(nc) as tc:
        with (
            tc.tile_pool(name="sb", bufs=1) as sb,
            tc.tile_pool(name="mm", bufs=2) as mm,
            tc.tile_pool(name="ps", bufs=2, space="PSUM") as ps,
        ):
            # --- wl/wr: contract W_fc[i, h*O+o] with attn_l/r[h, o] -> [i, 2H]
            wfc_t = [sb.tile([P, H * O], FP, tag=f"wfc{c}", name=f"wfc{c}") for c in range(2)]
            for c in range(2):
                nc.sync.dma_start(wfc_t[c][:], w_fc[c * P:(c + 1) * P, :])
            alr_t = sb.tile([P, 2 * H * O], FP)
            nc.sync.dma_start(alr_t[:], attn_lr[:])
            wlr = [sb.tile([P, 2 * H], FP, tag=f"wlr{c}", name=f"wlr{c}") for c in range(2)]
            for c in range(2):
                for half in range(2):  # 0: attn_l, 1: attn_r
                    tmp = mm.tile([P, H * O], FP, tag="wtmp")
                    nc.vector.tensor_tensor(
                        tmp[:], wfc_t[c][:],
                        alr_t[:, half * H * O:(half + 1) * H * O],
                        mybir.AluOpType.mult)
                    nc.vector.tensor_reduce(
                        wlr[c][:, half * H:(half + 1) * H],
                        tmp[:].rearrange("p (h o) -> p h o", h=H),
                        mybir.AxisListType.X, mybir.AluOpType.add)

            # --- ee table: (edge_emb @ W_e) [T, H*F] contract attn_e -> [T, H]
            embT_t = sb.tile([F, T], FP)
            nc.sync.dma_start(embT_t[:], edge_embT[:])
            we_t = sb.tile([F, H * F], FP)
            nc.sync.dma_start(we_t[:], w_e[:])
            ae_t = sb.tile([T, H * F], FP)
            nc.sync.dma_start(ae_t[:], attn_e[:])
            proj_ps = ps.tile([T, H * F], FP)
            nc.tensor.matmul(proj_ps[:], lhsT=embT_t[:], rhs=we_t[:],
                             start=True, stop=True)
            proj_sb = sb.tile([T, H * F], FP)
            nc.vector.tensor_tensor(
                proj_sb[:], proj_ps[:], ae_t[:],
                mybir.AluOpType.mult)
            ee_sb = sb.tile([T, H], FP)
            nc.vector.tensor_reduce(
                ee_sb[:], proj_sb[:].rearrange("t (h f) -> t h f", h=H),
                mybir.AxisListType.X, mybir.AluOpType.add)
            eep_sb = sb.tile([T, H], FP)
            nc.scalar.activation(eep_sb[:], ee_sb[:],
                                 mybir.ActivationFunctionType.Exp)
            nc.sync.dma_start(eep[:], eep_sb[:])

            # --- el/er for the shard: node ln = p*G + tt handled by
            #     (tile tt, psum partition p)
            ftT = [sb.tile([P, NSP], FP, tag=f"ft{c}", name=f"ft{c}") for c in range(2)]
            for c in range(2):
                nc.sync.dma_start(ftT[c][:], featT[c * P:(c + 1) * P, :])
            elr = sb.tile([P, G, 2 * H], FP)
            # batch 32 node-tiles per single-bank PSUM tile ([128, 512] f32);
            # accumulation stays strictly sequential per 16-col slice (the
            # HW-verified pattern) -- only the exp drain is batched per bank.
            SLICES = 32
            tt = 0
            while tt < G:
                nsl = min(SLICES, G - tt)
                bank = ps.tile([P, SLICES * 2 * H], FP, tag="bank")
                for j in range(nsl):
                    sl = bank[:, j * 2 * H:(j + 1) * 2 * H]
                    for c in range(2):
                        lhsT = ftT[c][:].rearrange("i (p t) -> i t p", p=P)[:, tt + j, :]
                        nc.tensor.matmul(sl, lhsT=lhsT, rhs=wlr[c][:],
                                         start=(c == 0), stop=(c == 1))
                nc.scalar.activation(
                    elr[:, tt:tt + nsl, :],
                    bank[:, :nsl * 2 * H].rearrange("p (t h) -> p t h", h=2 * H),
                    mybir.ActivationFunctionType.Exp)
                tt += nsl
            # write out: partition p holds nodes [G*p, G*(p+1))
            nc.sync.dma_start(
                erp[:].rearrange("(p t) h -> p t h", p=P), elr[:, :, H:2 * H])
            # el8 shard: row (ln*T + t) = el'[ln] * ee'[t]
            eeb = sb.tile([P, T * H], FP)
            nc.sync.dma_start(
                eeb[:],
                eep[:].rearrange("t h -> (t h)").unsqueeze(0)
                .to_broadcast([P, T * H]))
            blk = sb.tile([P, G, T, H], FP)
            nc.vector.tensor_tensor(
                blk[:],
                elr[:, :, 0:H].unsqueeze(2).to_broadcast([P, G, T, H]),
                eeb[:].rearrange("p (t h) -> p t h", t=T).unsqueeze(1)
                .to_broadcast([P, G, T, H]),
                mybir.AluOpType.mult)
            nc.sync.dma_start(
                el8s[:].rearrange("(p g t) h -> p g t h", p=P, t=T), blk[:])

    nc.compile()
    return nc


# ---------------------------------------------------------------------------
# Launch B: edge softmax
# ---------------------------------------------------------------------------

def _build_launch_b(gds, ktot):
    """gds: per-group slot width D_g (len G); ktot = sum(gds)."""
    nc = bacc.Bacc("TRN2", target_bir_lowering=False, debug=False,
                   num_devices=NCORES)
    el8 = nc.dram_tensor("el8", [EL8_ROWS, H], FP, kind="ExternalInput")
    er_grid = nc.dram_tensor("er_grid", [P, G * H], FP, kind="ExternalInput")
    deg = nc.dram_tensor("deg", [P, G], FP, kind="ExternalInput")
    idx = nc.dram_tensor("idx", [P, ktot], I32, kind="ExternalInput")
    out = nc.dram_tensor("out", [P, ktot * H], FP, kind="ExternalOutput")

    with tile.TileContext(nc) as tc:
        # gather + softmax chain, one group of 128 dst nodes at a time
        with (
            tc.tile_pool(name="cst", bufs=1) as cst,
            tc.tile_pool(name="gp", bufs=3) as gp,
            tc.tile_pool(name="yp", bufs=3) as yp,
            tc.tile_pool(name="ip", bufs=3) as ip,
            tc.tile_pool(name="sp", bufs=3) as sp,
        ):
            er_sb = cst.tile([P, G, H], FP)
            nc.sync.dma_start(er_sb[:],
                              er_grid[:].rearrange("p (g h) -> p g h", g=G))
            deg_sb = cst.tile([P, G], FP)
            nc.sync.dma_start(deg_sb[:], deg[:])

            k0 = 0
            for g in range(len(gds)):
                dd = gds[g]
                idx_t = ip.tile([P, dd], I32, tag="idx")
                nc.sync.dma_start(idx_t[:], idx[:, k0:k0 + dd])
                g_t = gp.tile([P, dd, H], FP, tag="g")
                for k in range(dd):
                    nc.gpsimd.indirect_dma_start(
                        out=g_t[:, k, :],
                        out_offset=None,
                        in_=el8[:],
                        in_offset=bass.IndirectOffsetOnAxis(
                            ap=idx_t[:, k:k + 1], axis=0),
                    )
                # m = g * er ; y = max(m - 1, 0)
                y_t = yp.tile([P, dd, H], FP, tag="y")
                nc.vector.tensor_tensor(
                    y_t[:], g_t[:],
                    er_sb[:, g, :].unsqueeze(1).to_broadcast([P, dd, H]),
                    mybir.AluOpType.mult)
                nc.vector.tensor_scalar(y_t[:], y_t[:], 1.0, 0.0,
                                        mybir.AluOpType.subtract,
                                        mybir.AluOpType.max)
                # s = sum_d y + deg ; r = 1/s
                sums = sp.tile([P, H], FP, tag="sums")
                nc.vector.tensor_reduce(
                    sums[:], y_t[:].rearrange("p d h -> p h d"),
                    mybir.AxisListType.X, mybir.AluOpType.add)
                s_t = sp.tile([P, H], FP, tag="s")
                nc.vector.tensor_tensor(
                    s_t[:], sums[:],
                    deg_sb[:, g:g + 1].to_broadcast([P, H]),
                    mybir.AluOpType.add)
                r_t = sp.tile([P, H], FP, tag="r")
                nc.vector.reciprocal(r_t[:], s_t[:])
                # out = (y + 1) * r   (into the gather tile, then store)
                nc.vector.scalar_tensor_tensor(
                    g_t[:], y_t[:], 1.0,
                    r_t[:].unsqueeze(1).to_broadcast([P, dd, H]),
                    mybir.AluOpType.add, mybir.AluOpType.mult)
                nc.sync.dma_start(
                    out[:, k0 * H:(k0 + dd) * H],
                    g_t[:].rearrange("p k h -> p (k h)"))
                k0 += dd

    nc.compile()
    return nc


# ---------------------------------------------------------------------------
# Host orchestration
# ---------------------------------------------------------------------------

def kernel(feat, etype, src, dst, W_fc, edge_emb, W_e, attn_l, attn_r, attn_e):
    feat = np.asarray(feat)
    etype = np.asarray(etype).astype(np.int64)
    src = np.asarray(src).astype(np.int64)
    dst = np.asarray(dst).astype(np.int64)
    W_fc = np.asarray(W_fc)
    edge_emb = np.asarray(edge_emb)
    W_e = np.asarray(W_e)
    attn_l = np.asarray(attn_l)
    attn_r = np.asarray(attn_r)
    attn_e = np.asarray(attn_e)

    # ---------------- Launch A ----------------
    nc_a = _build_launch_a()
    attn_lr = np.concatenate(
        [attn_l.reshape(1, H * O), attn_r.reshape(1, H * O)], axis=1)
    in_maps_a = []
    for s in range(NCORES):
        featT_s = np.zeros((IN, NSP), np.float32)
        featT_s[:, :NS] = feat[s * NS:(s + 1) * NS].T
        in_maps_a.append({
            "featT": featT_s,
            "w_fc": W_fc.astype(np.float32),
            "attn_lr": np.broadcast_to(attn_lr.astype(np.float32), (P, 2 * H * O)).copy(),
            "edge_embT": np.ascontiguousarray(edge_emb.T.astype(np.float32)),
            "w_e": W_e.astype(np.float32),
            "attn_e": np.broadcast_to(attn_e.reshape(1, H * F).astype(np.float32), (T, H * F)).copy(),
        })
    res_a = run_bass_kernel_spmd(nc_a, in_maps_a, core_ids=list(range(NCORES)))

    # el8 row(n, t) = (12544*(n//12500) + n%12500)*T + t ; tail rows zero pad
    el8_full = np.zeros((EL8_ROWS, H), np.float32)
    er_all = np.zeros((NCORES, NSP, H), np.float32)
    for s in range(NCORES):
        el8_full[s * NSP * T:(s + 1) * NSP * T] = res_a.results[s]["el8s"]
        er_all[s] = res_a.results[s]["erp"]

    # ---------------- host index construction (integers only) -------------
    # edges to cores by dst range; dst-sort within core
    core_of = dst // NS
    order_all = np.argsort(core_of * (2 * N) + dst, kind="stable")

    per_core = []
    for c in range(NCORES):
        lo = np.searchsorted(core_of[order_all], c, side="left")
        hi = np.searchsorted(core_of[order_all], c, side="right")
        per_core.append(order_all[lo:hi])

    # degree-sorted node grouping per core (shared chunk structure)
    node_perm = np.zeros((NCORES, NSP), np.int64)   # grid pos -> local node
    degrees = np.zeros((NCORES, NSP), np.int64)
    for c in range(NCORES):
        e_ids = per_core[c]
        ld = dst[e_ids] - c * NS
        cnt = np.bincount(ld, minlength=NSP)
        perm = np.argsort(cnt, kind="stable")       # ascending degree
        node_perm[c] = perm
        degrees[c] = cnt[perm]

    # groups: grid position (p, g) -> node_perm[g*128 + p]  (sorted order runs
    # down the group-axis first so consecutive groups have similar degrees)
    # group g covers sorted positions [g*128, (g+1)*128)
    gmax = degrees.reshape(NCORES, G, P).max(axis=2).max(axis=0)  # [G]

    # per-group slot width
    gds = [int(max(d, 1)) for d in gmax]
    ktot = sum(gds)

    nc_b = _build_launch_b(gds, ktot)

    # per-core B inputs
    in_maps_b = []
    slot_edge = np.full((NCORES, P, ktot), -1, np.int64)  # slot -> edge id
    for c in range(NCORES):
        e_ids = per_core[c]                      # dst-sorted edge ids
        ld = dst[e_ids] - c * NS
        cnt = np.bincount(ld, minlength=NSP)
        starts = np.concatenate([[0], np.cumsum(cnt)])
        perm = node_perm[c]
        inv_sorted_pos = np.empty(NSP, np.int64)
        inv_sorted_pos[perm] = np.arange(NSP)

        colbase = np.concatenate([[0], np.cumsum(gds)[:-1]]).astype(np.int64)

        nodes_pg = perm.reshape(G, P)                    # grid (g, p) -> node
        er_grid = er_all[c][nodes_pg].transpose(1, 0, 2)  # [P, G, H]
        deg_np = np.maximum(cnt[nodes_pg], 1).T.astype(np.float32)  # [P, G]

        # vectorized per-edge slot assignment (e_ids is dst-sorted)
        ld = dst[e_ids] - c * NS
        rank = np.arange(len(e_ids)) - starts[ld]
        spos = inv_sorted_pos[ld]
        gg_ = spos // P
        pp_ = spos % P
        cols = colbase[gg_] + rank
        rows = (src[e_ids] // NS) * NSP + (src[e_ids] % NS)
        idx_np = np.full((P, ktot), SENTINEL * T, np.int64)
        idx_np[pp_, cols] = rows * T + etype[e_ids]
        slot_edge[c, pp_, cols] = e_ids

        in_maps_b.append({
            "el8": el8_full,
            "er_grid": er_grid.reshape(P, G * H),
            "deg": deg_np,
            "idx": idx_np.astype(np.int32),
        })

    res_b = run_bass_kernel_spmd(nc_b, in_maps_b, core_ids=list(range(NCORES)))

    # ---------------- unshard ----------------
    out = np.zeros((E, H), np.float32)
    for c in range(NCORES):
        o_c = res_b.results[c]["out"].reshape(P, ktot, H)
        mask = slot_edge[c] >= 0
        out[slot_edge[c][mask]] = o_c[mask]

    # timing estimate via the cost-model simulator (no NTFF profiling
    # available under this axon client; see test.py)
    try:
        from concourse.timeline_sim import TimelineSim
        _timings["A_ns"] = TimelineSim(nc_a).simulate()
        _timings["B_ns"] = TimelineSim(nc_b).simulate()
    except Exception as ex:  # timing must never break correctness
        _timings["error"] = repr(ex)

    return out



# revision 5
# speedup vs baseline: 3.7181x; 3.7181x over previous
"""Trainium2 Bass kernel for nn_AttentionWeight (GAT edge softmax).

out[e,h] = softmax_over_dst_segments(relu(el[src]+er[dst]+ee[etype]))

Math used on device:
  exp(relu(x)) = max(exp(x), 1)  and  exp(x) = exp(el)*exp(ee)*exp(er)
  y := exp(relu(x)) - 1 = max(el'[src]*ee'[t]*er'[dst] - 1, 0)
  segment_sum(exp(relu(x))) = sum(y) + deg   (padding slots give y = 0)
  out = (y + 1) * reciprocal(segment_sum)

Distribution (8 NeuronCores, dst-range sharded):
  Launch A: node-sharded projections. Core s owns nodes [12500s, 12500(s+1)):
    el'/er' = exp(feat @ (W_fc contracted with attn_l/attn_r)); the tiny
    edge-type table ee' = exp(contract(edge_emb@W_e, attn_e)).
  Launch B: edges grouped by virtual row vr=(dst,etype), vrs count-sorted into
    [128 x Gv] grids with per-group slot width dd. One batched dma_gather per
    column batch fetches 256B rows of the el' block table (8 nodes/row,
    idx = src>>3 fits int16); an 8-way one-hot mask select (host-built integer
    masks) picks el'[src]; multiply by ee'[t]*er'[n] per vr; y = relu(m-1) on
    the scalar engine; per-vr partial sums via strided reduce. y goes to HBM
    in fp16; partial sums in f32.
  Launch C: per-node denominators: host permutes per-vr partials to node-major
    (indexing only), C sums over etypes, adds degree, reciprocal -> r.
  Launch D: host permutes r to vr-grid order (indexing only); D streams y,
    computes (y+1)*r, writes the slot values; host scatters to edge order.

All floating-point arithmetic happens on device; the host only shards,
permutes, concatenates and builds integer index/count/mask arrays.
"""

import sys

sys.path.insert(0, "/opt/trn_rl_repo")

import numpy as np

import concourse.bass as bass
import concourse.bacc as bacc
import concourse.mybir as mybir
import concourse.tile as tile
from concourse import library_config
from concourse.bass_utils import run_bass_kernel_spmd

# problem constants (hardcoded per harness contract)
N = 100000
E = 3200000
IN = 256
H = 8
O = 64
F = 64
T = 8
NCORES = 8
P = 128

NS = N // NCORES            # 12500 nodes per shard
NSP = 12544                 # padded to 128*98
GN = NSP // P               # 98 groups of 128 nodes (launch A/C grids)
NBLK = N // 8               # 12500 el' block-table rows (8 nodes x 8 heads)
COLS_MAX = 112              # max slot columns per gather batch (ring limit)

FP = mybir.dt.float32
F16 = mybir.dt.float16
I16 = mybir.dt.int16

_timings = {}


# ---------------------------------------------------------------------------
# Launch A: projections
# ---------------------------------------------------------------------------

def _build_launch_a():
    nc = bacc.Bacc("TRN2", target_bir_lowering=False, debug=False,
                   num_devices=NCORES)
    featT = nc.dram_tensor("featT", [IN, NSP], FP, kind="ExternalInput")
    w_fc = nc.dram_tensor("w_fc", [IN, H * O], FP, kind="ExternalInput")
    attn_lr = nc.dram_tensor("attn_lr", [P, 2 * H * O], FP, kind="ExternalInput")
    edge_embT = nc.dram_tensor("edge_embT", [F, T], FP, kind="ExternalInput")
    w_e = nc.dram_tensor("w_e", [F, H * F], FP, kind="ExternalInput")
    attn_e = nc.dram_tensor("attn_e", [T, H * F], FP, kind="ExternalInput")
    elp = nc.dram_tensor("elp", [NSP, H], FP, kind="ExternalOutput")
    erp = nc.dram_tensor("erp", [NSP, H], FP, kind="ExternalOutput")
    eep = nc.dram_tensor("eep", [T, H], FP, kind="ExternalOutput")

    with tile.TileContext(nc) as tc:
        with (
            tc.tile_pool(name="sb", bufs=1) as sb,
            tc.tile_pool(name="mm", bufs=2) as mm,
            tc.tile_pool(name="ps", bufs=2, space="PSUM") as ps,
        ):
            # --- wl/wr: contract W_fc[i, h*O+o] with attn_l/r[h, o] -> [i, 2H]
            wfc_t = [sb.tile([P, H * O], FP, tag=f"wfc{c}", name=f"wfc{c}") for c in range(2)]
            for c in range(2):
                nc.sync.dma_start(wfc_t[c][:], w_fc[c * P:(c + 1) * P, :])
            alr_t = sb.tile([P, 2 * H * O], FP)
            nc.sync.dma_start(alr_t[:], attn_lr[:])
            wlr = [sb.tile([P, 2 * H], FP, tag=f"wlr{c}", name=f"wlr{c}") for c in range(2)]
            for c in range(2):
                for half in range(2):  # 0: attn_l, 1: attn_r
                    tmp = mm.tile([P, H * O], FP, tag="wtmp")
                    nc.vector.tensor_tensor(
                        tmp[:], wfc_t[c][:],
                        alr_t[:, half * H * O:(half + 1) * H * O],
                        mybir.AluOpType.mult)
                    nc.vector.tensor_reduce(
                        wlr[c][:, half * H:(half + 1) * H],
                        tmp[:].rearrange("p (h o) -> p h o", h=H),
                        mybir.AxisListType.X, mybir.AluOpType.add)

            # --- ee table: (edge_emb @ W_e) [T, H*F] contract attn_e -> [T, H]
            embT_t = sb.tile([F, T], FP)
            nc.sync.dma_start(embT_t[:], edge_embT[:])
            we_t = sb.tile([F, H * F], FP)
            nc.sync.dma_start(we_t[:], w_e[:])
            ae_t = sb.tile([T, H * F], FP)
            nc.sync.dma_start(ae_t[:], attn_e[:])
            proj_ps = ps.tile([T, H * F], FP)
            nc.tensor.matmul(proj_ps[:], lhsT=embT_t[:], rhs=we_t[:],
                             start=True, stop=True)
            proj_sb = sb.tile([T, H * F], FP)
            nc.vector.tensor_tensor(
                proj_sb[:], proj_ps[:], ae_t[:],
                mybir.AluOpType.mult)
            ee_sb = sb.tile([T, H], FP)
            nc.vector.tensor_reduce(
                ee_sb[:], proj_sb[:].rearrange("t (h f) -> t h f", h=H),
                mybir.AxisListType.X, mybir.AluOpType.add)
            eep_sb = sb.tile([T, H], FP)
            nc.scalar.activation(eep_sb[:], ee_sb[:],
                                 mybir.ActivationFunctionType.Exp)
            nc.sync.dma_start(eep[:], eep_sb[:])

            # --- el/er for the shard: node ln = p*GN + tt handled by
            #     (tile tt, psum partition p)
            ftT = [sb.tile([P, NSP], FP, tag=f"ft{c}", name=f"ft{c}") for c in range(2)]
            for c in range(2):
                nc.sync.dma_start(ftT[c][:], featT[c * P:(c + 1) * P, :])
            elr = sb.tile([P, GN, 2 * H], FP)
            SLICES = 32
            tt = 0
            while tt < GN:
                nsl = min(SLICES, GN - tt)
                bank = ps.tile([P, SLICES * 2 * H], FP, tag="bank")
                for j in range(nsl):
                    sl = bank[:, j * 2 * H:(j + 1) * 2 * H]
                    for c in range(2):
                        lhsT = ftT[c][:].rearrange("i (p t) -> i t p", p=P)[:, tt + j, :]
                        nc.tensor.matmul(sl, lhsT=lhsT, rhs=wlr[c][:],
                                         start=(c == 0), stop=(c == 1))
                nc.scalar.activation(
                    elr[:, tt:tt + nsl, :],
                    bank[:, :nsl * 2 * H].rearrange("p (t h) -> p t h", h=2 * H),
                    mybir.ActivationFunctionType.Exp)
                tt += nsl
            # write out: partition p holds nodes [GN*p, GN*(p+1))
            nc.sync.dma_start(
                elp[:].rearrange("(p t) h -> p t h", p=P), elr[:, :, 0:H])
            nc.sync.dma_start(
                erp[:].rearrange("(p t) h -> p t h", p=P), elr[:, :, H:2 * H])

    nc.compile()
    return nc


# ---------------------------------------------------------------------------
# Launch B: gather + numerators + per-vr partial sums
# ---------------------------------------------------------------------------

def _build_launch_b(batches, Gv, ktot):
    """batches: list of (g0, n_g, dd, col0); Gv groups, ktot slot columns."""
    nc = bacc.Bacc("TRN2", target_bir_lowering=False, debug=False,
                   num_devices=NCORES, dynamic_dma_scratch_size=32768)
    table = nc.dram_tensor("table", [NBLK, 64], FP, kind="ExternalInput")
    idx_all = nc.dram_tensor("idx_all", [P, 8 * ktot], I16, kind="ExternalInput")
    mk_all = nc.dram_tensor("mk_all", [P, ktot * 8], FP, kind="ExternalInput")
    er_vr = nc.dram_tensor("er_vr", [P, Gv * H], FP, kind="ExternalInput")
    ee_vr = nc.dram_tensor("ee_vr", [P, Gv * H], FP, kind="ExternalInput")
    ps_out = nc.dram_tensor("ps_out", [P, Gv * H], FP, kind="ExternalOutput")
    y_out = nc.dram_tensor("y_out", [P, ktot * H], F16, kind="ExternalOutput")

    with tile.TileContext(nc) as tc:
        with (
            tc.tile_pool(name="cst", bufs=1) as cst,
            tc.tile_pool(name="gp", bufs=2) as gp,
            tc.tile_pool(name="ip", bufs=2) as ip,
            tc.tile_pool(name="tp", bufs=2) as tp,
            tc.tile_pool(name="sp", bufs=2) as sp,
            tc.tile_pool(name="yp", bufs=2) as yp,
            tc.tile_pool(name="pp", bufs=2) as pp,
        ):
            nc.gpsimd.load_library(library_config.mlp)
            bias_t = cst.tile([P, 1], FP)
            nc.vector.memset(bias_t[:], -1.0)

            for (g0, ng, dd, col0) in batches:
                cols = ng * dd
                ni = P * cols
                idx_t = ip.tile([P, 8 * cols], I16, tag="idx")
                nc.sync.dma_start(idx_t[:], idx_all[:, 8 * col0:8 * (col0 + cols)])
                mk_t = ip.tile([P, cols, 8], FP, tag="mk")
                nc.sync.dma_start(
                    mk_t[:],
                    mk_all[:, col0 * 8:(col0 + cols) * 8]
                    .rearrange("p (c k) -> p c k", k=8))
                erv_t = ip.tile([P, ng, H], FP, tag="erv")
                nc.sync.dma_start(
                    erv_t[:],
                    er_vr[:, g0 * H:(g0 + ng) * H].rearrange("p (g h) -> p g h", h=H))
                eev_t = ip.tile([P, ng, H], FP, tag="eev")
                nc.sync.dma_start(
                    eev_t[:],
                    ee_vr[:, g0 * H:(g0 + ng) * H].rearrange("p (g h) -> p g h", h=H))

                g_t = gp.tile([P, cols, 64], FP, tag="g")
                nc.gpsimd.dma_gather(g_t[:], table[:], idx_t[:], ni, ni, 64,
                                     single_packet=False)

                # vm = er' * ee' per vr
                vm_t = sp.tile([P, ng, H], FP, tag="vm")
                nc.vector.tensor_tensor(vm_t[:], erv_t[:], eev_t[:],
                                        mybir.AluOpType.mult)
                # tmp = g * one-hot(k) (mask broadcast over h)
                tmp_t = tp.tile([P, cols, 8, H], FP, tag="tmp")
                nc.vector.scalar_tensor_tensor(
                    tmp_t[:],
                    g_t[:].rearrange("p c (k h) -> p c k h", h=H),
                    1.0,
                    mk_t[:].unsqueeze(3).to_broadcast([P, cols, 8, H]),
                    mybir.AluOpType.mult,
                    mybir.AluOpType.mult,
                )
                # sel[p,c,h] = sum_k tmp
                sel_t = sp.tile([P, cols, H], FP, tag="sel")
                nc.vector.tensor_reduce(
                    sel_t[:],
                    tmp_t[:].rearrange("p c k h -> p c h k"),
                    mybir.AxisListType.X,
                    mybir.AluOpType.add,
                )
                # m = sel * vm (vm broadcast over dd)
                m_t = sp.tile([P, cols, H], FP, tag="m")
                nc.vector.tensor_tensor(
                    m_t[:].rearrange("p (g d) h -> p g d h", d=dd),
                    sel_t[:].rearrange("p (g d) h -> p g d h", d=dd),
                    vm_t[:].unsqueeze(2).to_broadcast([P, ng, dd, H]),
                    mybir.AluOpType.mult,
                )
                # y = relu(m - 1) on the scalar engine, cast to fp16
                y_t = yp.tile([P, cols, H], F16, tag="y")
                nc.scalar.activation(y_t[:], m_t[:],
                                     mybir.ActivationFunctionType.Relu,
                                     bias=bias_t[:])
                # ps[p,g,h] = sum_d y
                ps_t = pp.tile([P, ng, H], FP, tag="ps")
                nc.vector.tensor_reduce(
                    ps_t[:],
                    y_t[:].rearrange("p (g d) h -> p g h d", d=dd),
                    mybir.AxisListType.X,
                    mybir.AluOpType.add,
                )
                nc.sync.dma_start(
                    ps_out[:, g0 * H:(g0 + ng) * H]
                    .rearrange("p (g h) -> p g h", h=H), ps_t[:])
                nc.sync.dma_start(
                    y_out[:, col0 * H:(col0 + cols) * H]
                    .rearrange("p (c h) -> p c h", h=H), y_t[:])

    nc.compile()
    return nc


# ---------------------------------------------------------------------------
# Launch C: denominators
# ---------------------------------------------------------------------------

def _build_launch_c():
    nc = bacc.Bacc("TRN2", target_bir_lowering=False, debug=False,
                   num_devices=NCORES)
    psn = nc.dram_tensor("psn", [NSP, T * H], FP, kind="ExternalInput")
    deg = nc.dram_tensor("deg", [P, GN], FP, kind="ExternalInput")
    r_out = nc.dram_tensor("r_out", [NSP, H], FP, kind="ExternalOutput")

    with tile.TileContext(nc) as tc:
        with tc.tile_pool(name="sb", bufs=1) as sb:
            psn_t = sb.tile([P, GN, T, H], FP)
            nc.sync.dma_start(
                psn_t[:],
                psn[:].rearrange("(p q) (t h) -> p q t h", p=P, h=H))
            deg_t = sb.tile([P, GN], FP)
            nc.sync.dma_start(deg_t[:], deg[:])
            s_t = sb.tile([P, GN, H], FP)
            nc.vector.tensor_reduce(
                s_t[:],
                psn_t[:].rearrange("p q t h -> p q h t"),
                mybir.AxisListType.X,
                mybir.AluOpType.add,
            )
            nc.vector.tensor_tensor(
                s_t[:], s_t[:],
                deg_t[:].unsqueeze(2).to_broadcast([P, GN, H]),
                mybir.AluOpType.add)
            r_t = sb.tile([P, GN, H], FP)
            nc.vector.reciprocal(r_t[:], s_t[:])
            nc.sync.dma_start(
                r_out[:].rearrange("(p q) h -> p q h", p=P), r_t[:])

    nc.compile()
    return nc


# ---------------------------------------------------------------------------
# Launch D: rescale
# ---------------------------------------------------------------------------

def _build_launch_d(batches, Gv, ktot):
    nc = bacc.Bacc("TRN2", target_bir_lowering=False, debug=False,
                   num_devices=NCORES)
    y_in = nc.dram_tensor("y_in", [P, ktot * H], F16, kind="ExternalInput")
    r_vr = nc.dram_tensor("r_vr", [P, Gv * H], FP, kind="ExternalInput")
    out = nc.dram_tensor("out", [P, ktot * H], FP, kind="ExternalOutput")

    with tile.TileContext(nc) as tc:
        with (
            tc.tile_pool(name="yp", bufs=3) as yp,
            tc.tile_pool(name="rp", bufs=3) as rp,
            tc.tile_pool(name="op", bufs=3) as op,
        ):
            for (g0, ng, dd, col0) in batches:
                cols = ng * dd
                y_t = yp.tile([P, cols, H], F16, tag="y")
                nc.sync.dma_start(
                    y_t[:],
                    y_in[:, col0 * H:(col0 + cols) * H]
                    .rearrange("p (c h) -> p c h", h=H))
                r_t = rp.tile([P, ng, H], FP, tag="r")
                nc.sync.dma_start(
                    r_t[:],
                    r_vr[:, g0 * H:(g0 + ng) * H].rearrange("p (g h) -> p g h", h=H))
                rcol_t = rp.tile([P, cols, H], FP, tag="rcol")
                nc.vector.tensor_copy(
                    rcol_t[:].rearrange("p (g d) h -> p g d h", d=dd),
                    r_t[:].unsqueeze(2).to_broadcast([P, ng, dd, H]))
                o_t = op.tile([P, cols, H], FP, tag="o")
                # out = (y + 1) * r
                nc.vector.scalar_tensor_tensor(
                    o_t[:], y_t[:], 1.0, rcol_t[:],
                    mybir.AluOpType.add,
                    mybir.AluOpType.mult,
                )
                nc.sync.dma_start(
                    out[:, col0 * H:(col0 + cols) * H]
                    .rearrange("p (c h) -> p c h", h=H), o_t[:])

    nc.compile()
    return nc


# ---------------------------------------------------------------------------
# Host orchestration
# ---------------------------------------------------------------------------

def kernel(feat, etype, src, dst, W_fc, edge_emb, W_e, attn_l, attn_r, attn_e):
    feat = np.asarray(feat)
    etype = np.asarray(etype).astype(np.int64)
    src = np.asarray(src).astype(np.int64)
    dst = np.asarray(dst).astype(np.int64)
    W_fc = np.asarray(W_fc)
    edge_emb = np.asarray(edge_emb)
    W_e = np.asarray(W_e)
    attn_l = np.asarray(attn_l)
    attn_r = np.asarray(attn_r)
    attn_e = np.asarray(attn_e)

    # ---------------- Launch A ----------------
    nc_a = _build_launch_a()
    attn_lr = np.concatenate(
        [attn_l.reshape(1, H * O), attn_r.reshape(1, H * O)], axis=1)
    in_maps_a = []
    for s in range(NCORES):
        featT_s = np.zeros((IN, NSP), np.float32)
        featT_s[:, :NS] = feat[s * NS:(s + 1) * NS].T
        in_maps_a.append({
            "featT": featT_s,
            "w_fc": W_fc.astype(np.float32),
            "attn_lr": np.broadcast_to(attn_lr.astype(np.float32), (P, 2 * H * O)).copy(),
            "edge_embT": np.ascontiguousarray(edge_emb.T.astype(np.float32)),
            "w_e": W_e.astype(np.float32),
            "attn_e": np.broadcast_to(attn_e.reshape(1, H * F).astype(np.float32), (T, H * F)).copy(),
        })
    res_a = run_bass_kernel_spmd(nc_a, in_maps_a, core_ids=list(range(NCORES)))

    el_full = np.concatenate(
        [res_a.results[s]["elp"][:NS] for s in range(NCORES)])   # [N, H]
    er_full = np.concatenate(
        [res_a.results[s]["erp"][:NS] for s in range(NCORES)])   # [N, H]
    eep = res_a.results[0]["eep"]                                # [T, H]
    table = np.ascontiguousarray(el_full.reshape(NBLK, 64))

    # ---------------- host index construction (integers only) -------------
    key = dst * T + etype
    order = np.argsort(key, kind="stable")          # by dst, then etype
    dst_sorted = dst[order]
    core_bounds = np.searchsorted(dst_sorted, [c * NS for c in range(NCORES + 1)])

    # per-core vr lists (vr = (local dst, etype) with cnt >= 1), count-sorted
    pc = []
    for c in range(NCORES):
        lo, hi = core_bounds[c], core_bounds[c + 1]
        e_ids = order[lo:hi]
        keys_c = key[order[lo:hi]] - c * NS * T     # local n*T + t
        vr_keys, vr_start, vr_cnt = np.unique(
            keys_c, return_index=True, return_counts=True)
        perm = np.argsort(vr_cnt, kind="stable")    # ascending cnt
        inv = np.empty(len(perm), np.int64)
        inv[perm] = np.arange(len(perm))
        pc.append(dict(e_ids=e_ids, keys_c=keys_c, vr_keys=vr_keys,
                       vr_start=vr_start, vr_cnt=vr_cnt, perm=perm, inv=inv))

    NV = max(len(p["vr_keys"]) for p in pc)
    Gv = (NV + P - 1) // P

    # shared per-group dd (max over cores)
    dd_g = np.zeros(Gv, np.int64)
    for c in range(NCORES):
        cnt_sorted = pc[c]["vr_cnt"][pc[c]["perm"]]
        padded = np.zeros(Gv * P, np.int64)
        padded[:len(cnt_sorted)] = cnt_sorted
        dd_g = np.maximum(dd_g, padded.reshape(Gv, P).max(axis=1))

    # batches of consecutive groups padded to a uniform dd (ascending dd)
    batches = []            # (g0, n_g, dd, col0)
    colstart_g = np.zeros(Gv, np.int64)
    col0 = 0
    g = 0
    while g < Gv:
        if dd_g[g] == 0:
            g += 1
            continue
        g0 = g
        ng = 1
        ddb = int(dd_g[g])
        while (g0 + ng < Gv and dd_g[g0 + ng] > 0
               and (ng + 1) * max(ddb, int(dd_g[g0 + ng])) <= COLS_MAX):
            ddb = max(ddb, int(dd_g[g0 + ng]))
            ng += 1
        for j in range(ng):
            colstart_g[g0 + j] = col0 + j * ddb
        batches.append((g0, ng, ddb, col0))
        col0 += ng * ddb
        g = g0 + ng
    ktot = col0

    nc_b = _build_launch_b(batches, Gv, ktot)

    dd_of_g = np.zeros(Gv, np.int64)
    for (g0, ng, ddb, c0) in batches:
        dd_of_g[g0:g0 + ng] = ddb
    assert (dd_g <= dd_of_g).all(), "batch width below group degree"

    # per-core B inputs
    in_maps_b = []
    slot_edge = np.full((NCORES, P, ktot), -1, np.int64)
    vr_pos = []             # per core: (node_local, t, p, g) per vr
    for c in range(NCORES):
        d = pc[c]
        nvc = len(d["vr_keys"])
        sortpos = d["inv"]                     # vr i -> sorted position
        g_of_vr = sortpos // P
        p_of_vr = sortpos % P

        # per-edge slot assignment
        vi = np.repeat(np.arange(nvc), d["vr_cnt"])
        rank = np.arange(len(d["e_ids"])) - d["vr_start"][vi]
        pp_ = p_of_vr[vi]
        cols_ = colstart_g[g_of_vr[vi]] + rank
        srcs = src[d["e_ids"]]

        idx_grid = np.zeros((P, ktot), np.int16)
        idx_grid[pp_, cols_] = (srcs >> 3).astype(np.int16)
        mk_grid = np.zeros((P, ktot, 8), np.float32)
        mk_grid[pp_, cols_, srcs & 7] = 1.0
        slot_edge[c, pp_, cols_] = d["e_ids"]

        # wrapped idx: per batch, position i=(j*128+p) at [i%16, i//16], x8
        idx_all = np.zeros((P, 8 * ktot), np.int16)
        for (g0, ng, ddb, c0) in batches:
            cols = ng * ddb
            flat = idx_grid[:, c0:c0 + cols].T.reshape(-1)       # i = j*128+p
            w = flat.reshape(-1, 16).T                           # [16, ni/16]
            idx_all[:, 8 * c0:8 * (c0 + cols)] = np.tile(w, (8, 1))

        # per-vr er'/ee' grids
        nodes_l = d["vr_keys"] // T
        ts = d["vr_keys"] % T
        er_g = np.zeros((P, Gv, H), np.float32)
        ee_g = np.zeros((P, Gv, H), np.float32)
        er_g[p_of_vr, g_of_vr] = er_full[c * NS + nodes_l]
        ee_g[p_of_vr, g_of_vr] = eep[ts]
        vr_pos.append((nodes_l, ts, p_of_vr, g_of_vr))

        in_maps_b.append({
            "table": table,
            "idx_all": idx_all,
            "mk_all": mk_grid.reshape(P, ktot * 8),
            "er_vr": er_g.reshape(P, Gv * H),
            "ee_vr": ee_g.reshape(P, Gv * H),
        })

    res_b = run_bass_kernel_spmd(nc_b, in_maps_b, core_ids=list(range(NCORES)))

    # ---------------- Launch C ----------------
    nc_c = _build_launch_c()
    degs = np.bincount(dst, minlength=N)
    in_maps_c = []
    for c in range(NCORES):
        ps_c = res_b.results[c]["ps_out"].reshape(P, Gv, H)
        nodes_l, ts, p_v, g_v = vr_pos[c]
        psn = np.zeros((NSP, T, H), np.float32)
        psn[nodes_l, ts] = ps_c[p_v, g_v]
        deg_c = np.zeros(NSP, np.float32)
        deg_c[:NS] = degs[c * NS:(c + 1) * NS]
        deg_c = np.maximum(deg_c, 1.0)
        in_maps_c.append({
            "psn": psn.reshape(NSP, T * H),
            "deg": deg_c.reshape(P, GN),
        })
    res_c = run_bass_kernel_spmd(nc_c, in_maps_c, core_ids=list(range(NCORES)))

    # ---------------- Launch D ----------------
    nc_d = _build_launch_d(batches, Gv, ktot)
    in_maps_d = []
    for c in range(NCORES):
        r_c = res_c.results[c]["r_out"]        # [NSP, H] node-major
        nodes_l, ts, p_v, g_v = vr_pos[c]
        r_g = np.zeros((P, Gv, H), np.float32)
        r_g[p_v, g_v] = r_c[nodes_l]
        in_maps_d.append({
            "y_in": res_b.results[c]["y_out"],
            "r_vr": r_g.reshape(P, Gv * H),
        })
    res_d = run_bass_kernel_spmd(nc_d, in_maps_d, core_ids=list(range(NCORES)))

    # ---------------- unshard ----------------
    out = np.zeros((E, H), np.float32)
    for c in range(NCORES):
        o_c = res_d.results[c]["out"].reshape(P, ktot, H)
        mask = slot_edge[c] >= 0
        out[slot_edge[c][mask]] = o_c[mask]

    # timing estimate via the cost-model simulator
    try:
        from concourse.timeline_sim import TimelineSim
        _timings["A_ns"] = TimelineSim(nc_a).simulate()
        _timings["B_ns"] = TimelineSim(nc_b).simulate()
        _timings["C_ns"] = TimelineSim(nc_c).simulate()
        _timings["D_ns"] = TimelineSim(nc_d).simulate()
    except Exception as ex:  # timing must never break correctness
        _timings["error"] = repr(ex)

    return out


# revision 16
# speedup vs baseline: 3.7903x; 1.0194x over previous
"""Trainium2 Bass kernel for nn_AttentionWeight (GAT edge softmax).

out[e,h] = softmax_over_dst_segments(relu(el[src]+er[dst]+ee[etype]))

Math used on device:
  exp(relu(x)) = max(exp(x), 1)  and  exp(x) = exp(el)*exp(ee)*exp(er)
  y := exp(relu(x)) - 1 = max(el'[src]*ee'[t]*er'[dst] - 1, 0)
  segment_sum(exp(relu(x))) = sum(y) + deg   (padding slots give y = 0)
  out = (y + 1) * reciprocal(segment_sum)

Distribution (8 NeuronCores, dst-range sharded):
  Launch A: node-sharded projections. Core s owns nodes [12500s, 12500(s+1)):
    el'/er' = exp(feat @ (W_fc contracted with attn_l/attn_r)); the tiny
    edge-type table ee' = exp(contract(edge_emb@W_e, attn_e)).
  Launch B: edges grouped by virtual row vr=(dst,etype), vrs count-sorted into
    [128 x Gv] grids with per-group slot width dd. One batched dma_gather per
    column batch fetches 256B rows of the el' block table (8 nodes/row,
    idx = src>>3 fits int16); an 8-way one-hot mask select (host-built integer
    masks) picks el'[src]; multiply by ee'[t]*er'[n] per vr; y = relu(m-1) on
    the scalar engine; per-vr partial sums via strided reduce. y goes to HBM
    in fp16; partial sums in f32.
  Launch C: per-node denominators: host permutes per-vr partials to node-major
    (indexing only), C sums over etypes, adds degree, reciprocal -> r.
  Launch D: host permutes r to vr-grid order (indexing only); D streams y,
    computes (y+1)*r, writes the slot values; host scatters to edge order.

All floating-point arithmetic happens on device; the host only shards,
permutes, concatenates and builds integer index/count/mask arrays.
"""

import sys

sys.path.insert(0, "/opt/trn_rl_repo")

import numpy as np

import concourse.bass as bass
import concourse.bacc as bacc
import concourse.mybir as mybir
import concourse.tile as tile
from concourse import library_config
from concourse.bass_utils import run_bass_kernel_spmd

# problem constants (hardcoded per harness contract)
N = 100000
E = 3200000
IN = 256
H = 8
O = 64
F = 64
T = 8
NCORES = 8
P = 128

NS = N // NCORES            # 12500 nodes per shard
NSP = 12544                 # padded to 128*98
GN = NSP // P               # 98 groups of 128 nodes (launch A/C grids)
NBLK = N // 8               # 12500 el' block-table rows (8 nodes x 8 heads)
COLS_MAX = 112              # max slot columns per gather batch (ucode limit)

FP = mybir.dt.float32
F16 = mybir.dt.float16
I16 = mybir.dt.int16

_timings = {}


# ---------------------------------------------------------------------------
# Launch A: projections
# ---------------------------------------------------------------------------

def _build_launch_a():
    nc = bacc.Bacc("TRN2", target_bir_lowering=False, debug=False,
                   num_devices=NCORES)
    featT = nc.dram_tensor("featT", [IN, NSP], FP, kind="ExternalInput")
    w_fc = nc.dram_tensor("w_fc", [IN, H * O], FP, kind="ExternalInput")
    attn_lr = nc.dram_tensor("attn_lr", [P, 2 * H * O], FP, kind="ExternalInput")
    edge_embT = nc.dram_tensor("edge_embT", [F, T], FP, kind="ExternalInput")
    w_e = nc.dram_tensor("w_e", [F, H * F], FP, kind="ExternalInput")
    attn_e = nc.dram_tensor("attn_e", [T, H * F], FP, kind="ExternalInput")
    elp = nc.dram_tensor("elp", [NSP, H], FP, kind="ExternalOutput")
    erp = nc.dram_tensor("erp", [NSP, H], FP, kind="ExternalOutput")
    eep = nc.dram_tensor("eep", [T, H], FP, kind="ExternalOutput")

    with tile.TileContext(nc) as tc:
        with (
            tc.tile_pool(name="sb", bufs=1) as sb,
            tc.tile_pool(name="mm", bufs=2) as mm,
            tc.tile_pool(name="ps", bufs=2, space="PSUM") as ps,
        ):
            # --- wl/wr: contract W_fc[i, h*O+o] with attn_l/r[h, o] -> [i, 2H]
            wfc_t = [sb.tile([P, H * O], FP, tag=f"wfc{c}", name=f"wfc{c}") for c in range(2)]
            for c in range(2):
                nc.sync.dma_start(wfc_t[c][:], w_fc[c * P:(c + 1) * P, :])
            alr_t = sb.tile([P, 2 * H * O], FP)
            nc.sync.dma_start(alr_t[:], attn_lr[:])
            wlr = [sb.tile([P, 2 * H], FP, tag=f"wlr{c}", name=f"wlr{c}") for c in range(2)]
            for c in range(2):
                for half in range(2):  # 0: attn_l, 1: attn_r
                    tmp = mm.tile([P, H * O], FP, tag="wtmp")
                    nc.vector.tensor_tensor(
                        tmp[:], wfc_t[c][:],
                        alr_t[:, half * H * O:(half + 1) * H * O],
                        mybir.AluOpType.mult)
                    nc.vector.tensor_reduce(
                        wlr[c][:, half * H:(half + 1) * H],
                        tmp[:].rearrange("p (h o) -> p h o", h=H),
                        mybir.AxisListType.X, mybir.AluOpType.add)

            # --- ee table: (edge_emb @ W_e) [T, H*F] contract attn_e -> [T, H]
            embT_t = sb.tile([F, T], FP)
            nc.sync.dma_start(embT_t[:], edge_embT[:])
            we_t = sb.tile([F, H * F], FP)
            nc.sync.dma_start(we_t[:], w_e[:])
            ae_t = sb.tile([T, H * F], FP)
            nc.sync.dma_start(ae_t[:], attn_e[:])
            proj_ps = ps.tile([T, H * F], FP)
            nc.tensor.matmul(proj_ps[:], lhsT=embT_t[:], rhs=we_t[:],
                             start=True, stop=True)
            proj_sb = sb.tile([T, H * F], FP)
            nc.vector.tensor_tensor(
                proj_sb[:], proj_ps[:], ae_t[:],
                mybir.AluOpType.mult)
            ee_sb = sb.tile([T, H], FP)
            nc.vector.tensor_reduce(
                ee_sb[:], proj_sb[:].rearrange("t (h f) -> t h f", h=H),
                mybir.AxisListType.X, mybir.AluOpType.add)
            eep_sb = sb.tile([T, H], FP)
            nc.scalar.activation(eep_sb[:], ee_sb[:],
                                 mybir.ActivationFunctionType.Exp)
            nc.sync.dma_start(eep[:], eep_sb[:])

            # --- el/er for the shard: node ln = p*GN + tt handled by
            #     (tile tt, psum partition p)
            ftT = [sb.tile([P, NSP], FP, tag=f"ft{c}", name=f"ft{c}") for c in range(2)]
            for c in range(2):
                nc.sync.dma_start(ftT[c][:], featT[c * P:(c + 1) * P, :])
            elr = sb.tile([P, GN, 2 * H], FP)
            SLICES = 32
            tt = 0
            while tt < GN:
                nsl = min(SLICES, GN - tt)
                bank = ps.tile([P, SLICES * 2 * H], FP, tag="bank")
                for j in range(nsl):
                    sl = bank[:, j * 2 * H:(j + 1) * 2 * H]
                    for c in range(2):
                        lhsT = ftT[c][:].rearrange("i (p t) -> i t p", p=P)[:, tt + j, :]
                        nc.tensor.matmul(sl, lhsT=lhsT, rhs=wlr[c][:],
                                         start=(c == 0), stop=(c == 1))
                nc.scalar.activation(
                    elr[:, tt:tt + nsl, :],
                    bank[:, :nsl * 2 * H].rearrange("p (t h) -> p t h", h=2 * H),
                    mybir.ActivationFunctionType.Exp)
                tt += nsl
            # write out: partition p holds nodes [GN*p, GN*(p+1))
            nc.sync.dma_start(
                elp[:].rearrange("(p t) h -> p t h", p=P), elr[:, :, 0:H])
            nc.sync.dma_start(
                erp[:].rearrange("(p t) h -> p t h", p=P), elr[:, :, H:2 * H])

    nc.compile()
    return nc


# ---------------------------------------------------------------------------
# Launch B: gather + numerators + per-vr partial sums
# ---------------------------------------------------------------------------

def _build_launch_b(batches, Gv, ktot):
    """batches: list of (g0, n_g, dd, col0); Gv groups, ktot slot columns."""
    nc = bacc.Bacc("TRN2", target_bir_lowering=False, debug=False,
                   num_devices=NCORES, dynamic_dma_scratch_size=32768)
    table = nc.dram_tensor("table", [NBLK, 64], FP, kind="ExternalInput")
    idx_all = nc.dram_tensor("idx_all", [P, 8 * ktot], I16, kind="ExternalInput")
    kk_all = nc.dram_tensor("kk_all", [P, ktot], FP, kind="ExternalInput")
    iota = nc.dram_tensor("iota", [P, 8], FP, kind="ExternalInput")
    er_vr = nc.dram_tensor("er_vr", [P, Gv * H], FP, kind="ExternalInput")
    ee_vr = nc.dram_tensor("ee_vr", [P, Gv * H], FP, kind="ExternalInput")
    ps_out = nc.dram_tensor("ps_out", [P, Gv * H], FP, kind="ExternalOutput")
    y_out = nc.dram_tensor("y_out", [P, ktot * H], F16, kind="ExternalOutput")

    with tile.TileContext(nc) as tc:
        with (
            tc.tile_pool(name="cst", bufs=1) as cst,
            tc.tile_pool(name="gp", bufs=2) as gp,
            tc.tile_pool(name="ip", bufs=2) as ip,
            tc.tile_pool(name="tp", bufs=2) as tp,
            tc.tile_pool(name="sp", bufs=2) as sp,
            tc.tile_pool(name="yp", bufs=2) as yp,
            tc.tile_pool(name="pp", bufs=2) as pp,
        ):
            nc.gpsimd.load_library(library_config.mlp)
            bias_t = cst.tile([P, 1], FP)
            nc.vector.memset(bias_t[:], -1.0)
            iota_t = cst.tile([P, 8], FP)
            nc.sync.dma_start(iota_t[:], iota[:])

            for (g0, ng, dd, col0) in batches:
                cols = ng * dd
                ni = P * cols
                idx_t = ip.tile([P, 8 * cols], I16, tag="idx")
                nc.sync.dma_start(idx_t[:], idx_all[:, 8 * col0:8 * (col0 + cols)])
                kk_t = ip.tile([P, cols], FP, tag="kk")
                nc.sync.dma_start(kk_t[:], kk_all[:, col0:col0 + cols])
                erv_t = ip.tile([P, ng, H], FP, tag="erv")
                nc.sync.dma_start(
                    erv_t[:],
                    er_vr[:, g0 * H:(g0 + ng) * H].rearrange("p (g h) -> p g h", h=H))
                eev_t = ip.tile([P, ng, H], FP, tag="eev")
                nc.sync.dma_start(
                    eev_t[:],
                    ee_vr[:, g0 * H:(g0 + ng) * H].rearrange("p (g h) -> p g h", h=H))

                g_t = gp.tile([P, cols, 64], FP, tag="g")
                nc.gpsimd.dma_gather(g_t[:], table[:], idx_t[:], ni, ni, 64,
                                     single_packet=False)

                # vm = er' * ee' per vr
                vm_t = sp.tile([P, ng, H], FP, tag="vm")
                nc.vector.tensor_tensor(vm_t[:], erv_t[:], eev_t[:],
                                        mybir.AluOpType.mult)
                # one-hot(k) from kk vs iota (pad slots have kk = -1)
                mk_t = tp.tile([P, cols, 8], FP, tag="mk")
                nc.vector.tensor_tensor(
                    mk_t[:],
                    kk_t[:].unsqueeze(2).to_broadcast([P, cols, 8]),
                    iota_t[:].unsqueeze(1).to_broadcast([P, cols, 8]),
                    mybir.AluOpType.is_equal)
                # g <- g * one-hot(k) in place (mask broadcast over h)
                nc.vector.scalar_tensor_tensor(
                    g_t[:].rearrange("p c (k h) -> p c k h", h=H),
                    g_t[:].rearrange("p c (k h) -> p c k h", h=H),
                    1.0,
                    mk_t[:].unsqueeze(3).to_broadcast([P, cols, 8, H]),
                    mybir.AluOpType.mult,
                    mybir.AluOpType.mult,
                )
                # sel[p,c,h] = sum_k g
                sel_t = sp.tile([P, cols, H], FP, tag="sel")
                nc.vector.tensor_reduce(
                    sel_t[:],
                    g_t[:].rearrange("p c (k h) -> p c h k", h=H),
                    mybir.AxisListType.X,
                    mybir.AluOpType.add,
                )
                # sel <- sel * vm in place (vm broadcast over dd)
                nc.vector.tensor_tensor(
                    sel_t[:].rearrange("p (g d) h -> p g d h", d=dd),
                    sel_t[:].rearrange("p (g d) h -> p g d h", d=dd),
                    vm_t[:].unsqueeze(2).to_broadcast([P, ng, dd, H]),
                    mybir.AluOpType.mult,
                )
                # y = relu(sel - 1) on the scalar engine, cast to fp16
                y_t = yp.tile([P, cols, H], F16, tag="y")
                nc.scalar.activation(y_t[:], sel_t[:],
                                     mybir.ActivationFunctionType.Relu,
                                     bias=bias_t[:])
                # ps[p,g,h] = sum_d y
                ps_t = pp.tile([P, ng, H], FP, tag="ps")
                nc.vector.tensor_reduce(
                    ps_t[:],
                    y_t[:].rearrange("p (g d) h -> p g h d", d=dd),
                    mybir.AxisListType.X,
                    mybir.AluOpType.add,
                )
                nc.sync.dma_start(
                    ps_out[:, g0 * H:(g0 + ng) * H]
                    .rearrange("p (g h) -> p g h", h=H), ps_t[:])
                nc.sync.dma_start(
                    y_out[:, col0 * H:(col0 + cols) * H]
                    .rearrange("p (c h) -> p c h", h=H), y_t[:])

    nc.compile()
    return nc


# ---------------------------------------------------------------------------
# Launch C: denominators
# ---------------------------------------------------------------------------

def _build_launch_c():
    nc = bacc.Bacc("TRN2", target_bir_lowering=False, debug=False,
                   num_devices=NCORES)
    psn = nc.dram_tensor("psn", [NSP, T * H], FP, kind="ExternalInput")
    deg = nc.dram_tensor("deg", [P, GN], FP, kind="ExternalInput")
    r_out = nc.dram_tensor("r_out", [NSP, H], FP, kind="ExternalOutput")

    with tile.TileContext(nc) as tc:
        with tc.tile_pool(name="sb", bufs=1) as sb:
            psn_t = sb.tile([P, GN, T, H], FP)
            nc.sync.dma_start(
                psn_t[:],
                psn[:].rearrange("(p q) (t h) -> p q t h", p=P, h=H))
            deg_t = sb.tile([P, GN], FP)
            nc.sync.dma_start(deg_t[:], deg[:])
            s_t = sb.tile([P, GN, H], FP)
            nc.vector.tensor_reduce(
                s_t[:],
                psn_t[:].rearrange("p q t h -> p q h t"),
                mybir.AxisListType.X,
                mybir.AluOpType.add,
            )
            nc.vector.tensor_tensor(
                s_t[:], s_t[:],
                deg_t[:].unsqueeze(2).to_broadcast([P, GN, H]),
                mybir.AluOpType.add)
            r_t = sb.tile([P, GN, H], FP)
            nc.vector.reciprocal(r_t[:], s_t[:])
            nc.sync.dma_start(
                r_out[:].rearrange("(p q) h -> p q h", p=P), r_t[:])

    nc.compile()
    return nc


# ---------------------------------------------------------------------------
# Launch D: rescale
# ---------------------------------------------------------------------------

def _build_launch_d(batches, Gv, ktot):
    nc = bacc.Bacc("TRN2", target_bir_lowering=False, debug=False,
                   num_devices=NCORES)
    y_in = nc.dram_tensor("y_in", [P, ktot * H], F16, kind="ExternalInput")
    r_vr = nc.dram_tensor("r_vr", [P, Gv * H], FP, kind="ExternalInput")
    out = nc.dram_tensor("out", [P, ktot * H], F16, kind="ExternalOutput")

    with tile.TileContext(nc) as tc:
        with (
            tc.tile_pool(name="yp", bufs=3) as yp,
            tc.tile_pool(name="rp", bufs=3) as rp,
            tc.tile_pool(name="op", bufs=3) as op,
        ):
            for (g0, ng, dd, col0) in batches:
                cols = ng * dd
                y_t = yp.tile([P, cols, H], F16, tag="y")
                nc.sync.dma_start(
                    y_t[:],
                    y_in[:, col0 * H:(col0 + cols) * H]
                    .rearrange("p (c h) -> p c h", h=H))
                r_t = rp.tile([P, ng, H], FP, tag="r")
                nc.sync.dma_start(
                    r_t[:],
                    r_vr[:, g0 * H:(g0 + ng) * H].rearrange("p (g h) -> p g h", h=H))
                rcol_t = rp.tile([P, cols, H], FP, tag="rcol")
                nc.gpsimd.tensor_copy(
                    rcol_t[:].rearrange("p (g d) h -> p g d h", d=dd),
                    r_t[:].unsqueeze(2).to_broadcast([P, ng, dd, H]))
                o_t = op.tile([P, cols, H], F16, tag="o")
                # out = (y + 1) * r
                nc.vector.scalar_tensor_tensor(
                    o_t[:], y_t[:], 1.0, rcol_t[:],
                    mybir.AluOpType.add,
                    mybir.AluOpType.mult,
                )
                nc.sync.dma_start(
                    out[:, col0 * H:(col0 + cols) * H]
                    .rearrange("p (c h) -> p c h", h=H), o_t[:])

    nc.compile()
    return nc


# ---------------------------------------------------------------------------
# Host orchestration
# ---------------------------------------------------------------------------

def kernel(feat, etype, src, dst, W_fc, edge_emb, W_e, attn_l, attn_r, attn_e):
    feat = np.asarray(feat)
    etype = np.asarray(etype).astype(np.int64)
    src = np.asarray(src).astype(np.int64)
    dst = np.asarray(dst).astype(np.int64)
    W_fc = np.asarray(W_fc)
    edge_emb = np.asarray(edge_emb)
    W_e = np.asarray(W_e)
    attn_l = np.asarray(attn_l)
    attn_r = np.asarray(attn_r)
    attn_e = np.asarray(attn_e)

    # ---------------- Launch A ----------------
    nc_a = _build_launch_a()
    attn_lr = np.concatenate(
        [attn_l.reshape(1, H * O), attn_r.reshape(1, H * O)], axis=1)
    in_maps_a = []
    for s in range(NCORES):
        featT_s = np.zeros((IN, NSP), np.float32)
        featT_s[:, :NS] = feat[s * NS:(s + 1) * NS].T
        in_maps_a.append({
            "featT": featT_s,
            "w_fc": W_fc.astype(np.float32),
            "attn_lr": np.broadcast_to(attn_lr.astype(np.float32), (P, 2 * H * O)).copy(),
            "edge_embT": np.ascontiguousarray(edge_emb.T.astype(np.float32)),
            "w_e": W_e.astype(np.float32),
            "attn_e": np.broadcast_to(attn_e.reshape(1, H * F).astype(np.float32), (T, H * F)).copy(),
        })
    res_a = run_bass_kernel_spmd(nc_a, in_maps_a, core_ids=list(range(NCORES)))

    el_full = np.concatenate(
        [res_a.results[s]["elp"][:NS] for s in range(NCORES)])   # [N, H]
    er_full = np.concatenate(
        [res_a.results[s]["erp"][:NS] for s in range(NCORES)])   # [N, H]
    eep = res_a.results[0]["eep"]                                # [T, H]
    table = np.ascontiguousarray(el_full.reshape(NBLK, 64))

    # ---------------- host index construction (integers only) -------------
    key = dst * T + etype
    order = np.argsort(key, kind="stable")          # by dst, then etype
    dst_sorted = dst[order]
    core_bounds = np.searchsorted(dst_sorted, [c * NS for c in range(NCORES + 1)])

    # per-core vr lists (vr = (local dst, etype) with cnt >= 1), count-sorted
    pc = []
    for c in range(NCORES):
        lo, hi = core_bounds[c], core_bounds[c + 1]
        e_ids = order[lo:hi]
        keys_c = key[order[lo:hi]] - c * NS * T     # local n*T + t
        vr_keys, vr_start, vr_cnt = np.unique(
            keys_c, return_index=True, return_counts=True)
        perm = np.argsort(vr_cnt, kind="stable")    # ascending cnt
        inv = np.empty(len(perm), np.int64)
        inv[perm] = np.arange(len(perm))
        pc.append(dict(e_ids=e_ids, keys_c=keys_c, vr_keys=vr_keys,
                       vr_start=vr_start, vr_cnt=vr_cnt, perm=perm, inv=inv))

    NV = max(len(p["vr_keys"]) for p in pc)
    Gv = (NV + P - 1) // P

    # shared per-group dd (max over cores)
    dd_g = np.zeros(Gv, np.int64)
    for c in range(NCORES):
        cnt_sorted = pc[c]["vr_cnt"][pc[c]["perm"]]
        padded = np.zeros(Gv * P, np.int64)
        padded[:len(cnt_sorted)] = cnt_sorted
        dd_g = np.maximum(dd_g, padded.reshape(Gv, P).max(axis=1))

    # batches of consecutive groups padded to a uniform dd (ascending dd)
    batches = []            # (g0, n_g, dd, col0)
    colstart_g = np.zeros(Gv, np.int64)
    col0 = 0
    g = 0
    while g < Gv:
        if dd_g[g] == 0:
            g += 1
            continue
        g0 = g
        ng = 1
        ddb = int(dd_g[g])
        while (g0 + ng < Gv and dd_g[g0 + ng] > 0
               and (ng + 1) * max(ddb, int(dd_g[g0 + ng])) <= COLS_MAX):
            ddb = max(ddb, int(dd_g[g0 + ng]))
            ng += 1
        for j in range(ng):
            colstart_g[g0 + j] = col0 + j * ddb
        batches.append((g0, ng, ddb, col0))
        col0 += ng * ddb
        g = g0 + ng
    ktot = col0

    nc_b = _build_launch_b(batches, Gv, ktot)

    dd_of_g = np.zeros(Gv, np.int64)
    for (g0, ng, ddb, c0) in batches:
        dd_of_g[g0:g0 + ng] = ddb
    assert (dd_g <= dd_of_g).all(), "batch width below group degree"

    # per-core B inputs
    in_maps_b = []
    slot_edge = np.full((NCORES, P, ktot), -1, np.int64)
    vr_pos = []             # per core: (node_local, t, p, g) per vr
    for c in range(NCORES):
        d = pc[c]
        nvc = len(d["vr_keys"])
        sortpos = d["inv"]                     # vr i -> sorted position
        g_of_vr = sortpos // P
        p_of_vr = sortpos % P

        # per-edge slot assignment
        vi = np.repeat(np.arange(nvc), d["vr_cnt"])
        rank = np.arange(len(d["e_ids"])) - d["vr_start"][vi]
        pp_ = p_of_vr[vi]
        cols_ = colstart_g[g_of_vr[vi]] + rank
        srcs = src[d["e_ids"]]

        idx_grid = np.zeros((P, ktot), np.int16)
        idx_grid[pp_, cols_] = (srcs >> 3).astype(np.int16)
        kk_grid = np.full((P, ktot), -1.0, np.float32)
        kk_grid[pp_, cols_] = (srcs & 7).astype(np.float32)
        slot_edge[c, pp_, cols_] = d["e_ids"]

        # wrapped idx: per batch, position i=(j*128+p) at [i%16, i//16], x8
        idx_all = np.zeros((P, 8 * ktot), np.int16)
        for (g0, ng, ddb, c0) in batches:
            cols = ng * ddb
            flat = idx_grid[:, c0:c0 + cols].T.reshape(-1)       # i = j*128+p
            w = flat.reshape(-1, 16).T                           # [16, ni/16]
            idx_all[:, 8 * c0:8 * (c0 + cols)] = np.tile(w, (8, 1))

        # per-vr er'/ee' grids
        nodes_l = d["vr_keys"] // T
        ts = d["vr_keys"] % T
        er_g = np.zeros((P, Gv, H), np.float32)
        ee_g = np.zeros((P, Gv, H), np.float32)
        er_g[p_of_vr, g_of_vr] = er_full[c * NS + nodes_l]
        ee_g[p_of_vr, g_of_vr] = eep[ts]
        vr_pos.append((nodes_l, ts, p_of_vr, g_of_vr))

        in_maps_b.append({
            "table": table,
            "idx_all": idx_all,
            "kk_all": kk_grid,
            "iota": np.broadcast_to(
                np.arange(8, dtype=np.float32), (P, 8)).copy(),
            "er_vr": er_g.reshape(P, Gv * H),
            "ee_vr": ee_g.reshape(P, Gv * H),
        })

    res_b = run_bass_kernel_spmd(nc_b, in_maps_b, core_ids=list(range(NCORES)))

    # ---------------- Launch C ----------------
    nc_c = _build_launch_c()
    degs = np.bincount(dst, minlength=N)
    in_maps_c = []
    for c in range(NCORES):
        ps_c = res_b.results[c]["ps_out"].reshape(P, Gv, H)
        nodes_l, ts, p_v, g_v = vr_pos[c]
        psn = np.zeros((NSP, T, H), np.float32)
        psn[nodes_l, ts] = ps_c[p_v, g_v]
        deg_c = np.zeros(NSP, np.float32)
        deg_c[:NS] = degs[c * NS:(c + 1) * NS]
        deg_c = np.maximum(deg_c, 1.0)
        in_maps_c.append({
            "psn": psn.reshape(NSP, T * H),
            "deg": deg_c.reshape(P, GN),
        })
    res_c = run_bass_kernel_spmd(nc_c, in_maps_c, core_ids=list(range(NCORES)))

    # ---------------- Launch D ----------------
    nc_d = _build_launch_d(batches, Gv, ktot)
    in_maps_d = []
    for c in range(NCORES):
        r_c = res_c.results[c]["r_out"]        # [NSP, H] node-major
        nodes_l, ts, p_v, g_v = vr_pos[c]
        r_g = np.zeros((P, Gv, H), np.float32)
        r_g[p_v, g_v] = r_c[nodes_l]
        in_maps_d.append({
            "y_in": res_b.results[c]["y_out"],
            "r_vr": r_g.reshape(P, Gv * H),
        })
    res_d = run_bass_kernel_spmd(nc_d, in_maps_d, core_ids=list(range(NCORES)))

    # ---------------- unshard ----------------
    out = np.zeros((E, H), np.float32)
    for c in range(NCORES):
        o_c = res_d.results[c]["out"].reshape(P, ktot, H).astype(np.float32)
        mask = slot_edge[c] >= 0
        out[slot_edge[c][mask]] = o_c[mask]

    # timing estimate via the cost-model simulator
    try:
        from concourse.timeline_sim import TimelineSim
        _timings["A_ns"] = TimelineSim(nc_a).simulate()
        _timings["B_ns"] = TimelineSim(nc_b).simulate()
        _timings["C_ns"] = TimelineSim(nc_c).simulate()
        _timings["D_ns"] = TimelineSim(nc_d).simulate()
    except Exception as ex:  # timing must never break correctness
        _timings["error"] = repr(ex)

    return out


# revision 33
# speedup vs baseline: 3.9114x; 1.0320x over previous
"""Trainium2 Bass kernel for nn_AttentionWeight (GAT edge softmax).

out[e,h] = softmax_over_dst_segments(relu(el[src]+er[dst]+ee[etype]))

Math used on device:
  exp(relu(x)) = max(exp(x), 1)  and  exp(x) = exp(el)*exp(ee)*exp(er)
  y := exp(relu(x)) - 1 = max(el'[src]*ee'[t]*er'[dst] - 1, 0)
  segment_sum(exp(relu(x))) = sum(y) + deg   (padding slots give y = 0)
  out = (y + 1) * reciprocal(segment_sum)

Distribution (8 NeuronCores, dst-range sharded):
  Launch A: node-sharded projections. Core s owns nodes [12500s, 12500(s+1)):
    el'/er' = exp(feat @ (W_fc contracted with attn_l/attn_r)); the tiny
    edge-type table ee' = exp(contract(edge_emb@W_e, attn_e)).
  Launch B: edges grouped by virtual row vr=(dst,etype), vrs count-sorted into
    [128 x Gv] grids with per-group slot width dd. One batched dma_gather per
    column batch fetches 256B rows of the el' block table (8 nodes/row,
    idx = src>>3 fits int16); an 8-way one-hot mask select (host-built integer
    masks) picks el'[src]; multiply by ee'[t]*er'[n] per vr; y = relu(m-1) on
    the scalar engine; per-vr partial sums via strided reduce. y goes to HBM
    in fp16; partial sums in f32.
  Launch C: per-node denominators: host permutes per-vr partials to node-major
    (indexing only), C sums over etypes, adds degree, reciprocal -> r.
  Launch D: host permutes r to vr-grid order (indexing only); D streams y,
    computes (y+1)*r, writes the slot values; host scatters to edge order.

All floating-point arithmetic happens on device; the host only shards,
permutes, concatenates and builds integer index/count/mask arrays.
"""

import sys

sys.path.insert(0, "/opt/trn_rl_repo")

import numpy as np

import concourse.bass as bass
import concourse.bacc as bacc
import concourse.mybir as mybir
import concourse.tile as tile
from concourse import library_config
from concourse.bass_utils import run_bass_kernel_spmd

# problem constants (hardcoded per harness contract)
N = 100000
E = 3200000
IN = 256
H = 8
O = 64
F = 64
T = 8
NCORES = 8
P = 128

NS = N // NCORES            # 12500 nodes per shard
NSP = 12544                 # padded to 128*98
GN = NSP // P               # 98 groups of 128 nodes (launch A/C grids)
NBLK = N // 8               # 12500 el' block-table rows (8 nodes x 8 heads)
COLS_MAX = 112              # max slot columns per gather batch (ucode limit)

FP = mybir.dt.float32
F16 = mybir.dt.float16
BF16 = mybir.dt.bfloat16
I16 = mybir.dt.int16

_timings = {}


# ---------------------------------------------------------------------------
# Launch A: projections
# ---------------------------------------------------------------------------

def _build_launch_a():
    nc = bacc.Bacc("TRN2", target_bir_lowering=False, debug=False,
                   num_devices=NCORES)
    featT = nc.dram_tensor("featT", [IN, NSP], BF16, kind="ExternalInput")
    w_fc = nc.dram_tensor("w_fc", [IN, H * O], FP, kind="ExternalInput")
    attn_lr = nc.dram_tensor("attn_lr", [P, 2 * H * O], FP, kind="ExternalInput")
    edge_embT = nc.dram_tensor("edge_embT", [F, T], FP, kind="ExternalInput")
    w_e = nc.dram_tensor("w_e", [F, H * F], FP, kind="ExternalInput")
    attn_e = nc.dram_tensor("attn_e", [T, H * F], FP, kind="ExternalInput")
    elp = nc.dram_tensor("elp", [NSP, H], FP, kind="ExternalOutput")
    erp = nc.dram_tensor("erp", [NSP, H], FP, kind="ExternalOutput")
    eep = nc.dram_tensor("eep", [T, H], FP, kind="ExternalOutput")

    with tile.TileContext(nc) as tc:
        with (
            tc.tile_pool(name="sb", bufs=1) as sb,
            tc.tile_pool(name="mm", bufs=2) as mm,
            tc.tile_pool(name="ps", bufs=2, space="PSUM") as ps,
        ):
            # --- wl/wr: contract W_fc[i, h*O+o] with attn_l/r[h, o] -> [i, 2H]
            wfc_t = [sb.tile([P, H * O], FP, tag=f"wfc{c}", name=f"wfc{c}") for c in range(2)]
            for c in range(2):
                nc.sync.dma_start(wfc_t[c][:], w_fc[c * P:(c + 1) * P, :])
            alr_t = sb.tile([P, 2 * H * O], FP)
            nc.sync.dma_start(alr_t[:], attn_lr[:])
            wlr = [sb.tile([P, 2 * H], FP, tag=f"wlr{c}", name=f"wlr{c}") for c in range(2)]
            for c in range(2):
                for half in range(2):  # 0: attn_l, 1: attn_r
                    tmp = mm.tile([P, H * O], FP, tag="wtmp")
                    nc.vector.tensor_tensor(
                        tmp[:], wfc_t[c][:],
                        alr_t[:, half * H * O:(half + 1) * H * O],
                        mybir.AluOpType.mult)
                    nc.vector.tensor_reduce(
                        wlr[c][:, half * H:(half + 1) * H],
                        tmp[:].rearrange("p (h o) -> p h o", h=H),
                        mybir.AxisListType.X, mybir.AluOpType.add)

            # --- ee table: (edge_emb @ W_e) [T, H*F] contract attn_e -> [T, H]
            embT_t = sb.tile([F, T], FP)
            nc.sync.dma_start(embT_t[:], edge_embT[:])
            we_t = sb.tile([F, H * F], FP)
            nc.sync.dma_start(we_t[:], w_e[:])
            ae_t = sb.tile([T, H * F], FP)
            nc.sync.dma_start(ae_t[:], attn_e[:])
            proj_ps = ps.tile([T, H * F], FP)
            nc.tensor.matmul(proj_ps[:], lhsT=embT_t[:], rhs=we_t[:],
                             start=True, stop=True)
            proj_sb = sb.tile([T, H * F], FP)
            nc.vector.tensor_tensor(
                proj_sb[:], proj_ps[:], ae_t[:],
                mybir.AluOpType.mult)
            ee_sb = sb.tile([T, H], FP)
            nc.vector.tensor_reduce(
                ee_sb[:], proj_sb[:].rearrange("t (h f) -> t h f", h=H),
                mybir.AxisListType.X, mybir.AluOpType.add)
            eep_sb = sb.tile([T, H], FP)
            nc.scalar.activation(eep_sb[:], ee_sb[:],
                                 mybir.ActivationFunctionType.Exp)
            nc.sync.dma_start(eep[:], eep_sb[:])

            # --- el/er for the shard: node ln = p*GN + tt handled by
            #     (tile tt, psum partition p)
            wlr_bf = [sb.tile([P, 2 * H], BF16, tag=f"wlrb{c}", name=f"wlrb{c}")
                      for c in range(2)]
            for c in range(2):
                nc.vector.tensor_copy(wlr_bf[c][:], wlr[c][:])
            ftT = [sb.tile([P, NSP], BF16, tag=f"ft{c}", name=f"ft{c}") for c in range(2)]
            for c in range(2):
                nc.sync.dma_start(ftT[c][:], featT[c * P:(c + 1) * P, :])
            elr = sb.tile([P, GN, 2 * H], FP)
            SLICES = 32
            tt = 0
            while tt < GN:
                nsl = min(SLICES, GN - tt)
                bank = ps.tile([P, SLICES * 2 * H], FP, tag="bank")
                for j in range(nsl):
                    sl = bank[:, j * 2 * H:(j + 1) * 2 * H]
                    for c in range(2):
                        lhsT = ftT[c][:].rearrange("i (p t) -> i t p", p=P)[:, tt + j, :]
                        nc.tensor.matmul(sl, lhsT=lhsT, rhs=wlr_bf[c][:],
                                         start=(c == 0), stop=(c == 1))
                nc.scalar.activation(
                    elr[:, tt:tt + nsl, :],
                    bank[:, :nsl * 2 * H].rearrange("p (t h) -> p t h", h=2 * H),
                    mybir.ActivationFunctionType.Exp)
                tt += nsl
            # write out: partition p holds nodes [GN*p, GN*(p+1))
            nc.sync.dma_start(
                elp[:].rearrange("(p t) h -> p t h", p=P), elr[:, :, 0:H])
            nc.sync.dma_start(
                erp[:].rearrange("(p t) h -> p t h", p=P), elr[:, :, H:2 * H])

    nc.compile()
    return nc


# ---------------------------------------------------------------------------
# Launch B: gather + numerators + per-vr partial sums
# ---------------------------------------------------------------------------

def _build_launch_b(batches, Gv, ktot):
    """batches: list of (g0, n_g, dd, col0); Gv groups, ktot slot columns."""
    nc = bacc.Bacc("TRN2", target_bir_lowering=False, debug=False,
                   num_devices=NCORES, dynamic_dma_scratch_size=32768)
    table = nc.dram_tensor("table", [NBLK, 64], FP, kind="ExternalInput")
    idx_all = nc.dram_tensor("idx_all", [P, 8 * ktot], I16, kind="ExternalInput")
    kk_all = nc.dram_tensor("kk_all", [P, ktot], F16, kind="ExternalInput")
    iota = nc.dram_tensor("iota", [P, 8], F16, kind="ExternalInput")
    er_vr = nc.dram_tensor("er_vr", [P, Gv * H], F16, kind="ExternalInput")
    ee_vr = nc.dram_tensor("ee_vr", [P, Gv * H], F16, kind="ExternalInput")
    ps_out = nc.dram_tensor("ps_out", [P, Gv * H], FP, kind="ExternalOutput")
    y_out = nc.dram_tensor("y_out", [P, ktot * H], F16, kind="ExternalOutput")

    with tile.TileContext(nc) as tc:
        with (
            tc.tile_pool(name="cst", bufs=1) as cst,
            tc.tile_pool(name="gp", bufs=2) as gp,
            tc.tile_pool(name="ip", bufs=2) as ip,
            tc.tile_pool(name="tp", bufs=2) as tp,
            tc.tile_pool(name="sp", bufs=2) as sp,
            tc.tile_pool(name="yp", bufs=2) as yp,
            tc.tile_pool(name="pp", bufs=2) as pp,
        ):
            nc.gpsimd.load_library(library_config.mlp)
            bias_t = cst.tile([P, 1], FP)
            nc.vector.memset(bias_t[:], -1.0)
            iota_t = cst.tile([P, 8], F16)
            nc.sync.dma_start(iota_t[:], iota[:])

            for (g0, ng, dd, col0) in batches:
                cols = ng * dd
                ni = P * cols
                idx_t = ip.tile([P, 8 * cols], I16, tag="idx")
                nc.sync.dma_start(idx_t[:], idx_all[:, 8 * col0:8 * (col0 + cols)])
                kk_t = ip.tile([P, cols], F16, tag="kk")
                nc.sync.dma_start(kk_t[:], kk_all[:, col0:col0 + cols])
                erv_t = ip.tile([P, ng, H], F16, tag="erv")
                nc.sync.dma_start(
                    erv_t[:],
                    er_vr[:, g0 * H:(g0 + ng) * H].rearrange("p (g h) -> p g h", h=H))
                eev_t = ip.tile([P, ng, H], F16, tag="eev")
                nc.sync.dma_start(
                    eev_t[:],
                    ee_vr[:, g0 * H:(g0 + ng) * H].rearrange("p (g h) -> p g h", h=H))

                g_t = gp.tile([P, cols, 64], FP, tag="g")
                nc.gpsimd.dma_gather(g_t[:], table[:], idx_t[:], ni, ni, 64,
                                     single_packet=False)

                # vm = er' * ee' per vr
                vm_t = sp.tile([P, ng, H], FP, tag="vm")
                nc.vector.tensor_tensor(vm_t[:], erv_t[:], eev_t[:],
                                        mybir.AluOpType.mult)
                # one-hot(k) from kk vs iota (pad slots have kk = -1)
                mk_t = tp.tile([P, cols, 8], FP, tag="mk")
                nc.vector.tensor_tensor(
                    mk_t[:],
                    kk_t[:].unsqueeze(2).to_broadcast([P, cols, 8]),
                    iota_t[:].unsqueeze(1).to_broadcast([P, cols, 8]),
                    mybir.AluOpType.is_equal)
                # g <- g * one-hot(k) in place (mask broadcast over h)
                nc.vector.scalar_tensor_tensor(
                    g_t[:].rearrange("p c (k h) -> p c k h", h=H),
                    g_t[:].rearrange("p c (k h) -> p c k h", h=H),
                    1.0,
                    mk_t[:].unsqueeze(3).to_broadcast([P, cols, 8, H]),
                    mybir.AluOpType.mult,
                    mybir.AluOpType.mult,
                )
                # sel[p,c,h] = sum_k g
                sel_t = sp.tile([P, cols, H], FP, tag="sel")
                nc.vector.tensor_reduce(
                    sel_t[:],
                    g_t[:].rearrange("p c (k h) -> p c h k", h=H),
                    mybir.AxisListType.X,
                    mybir.AluOpType.add,
                )
                # sel <- sel * vm in place (vm broadcast over dd)
                nc.vector.tensor_tensor(
                    sel_t[:].rearrange("p (g d) h -> p g d h", d=dd),
                    sel_t[:].rearrange("p (g d) h -> p g d h", d=dd),
                    vm_t[:].unsqueeze(2).to_broadcast([P, ng, dd, H]),
                    mybir.AluOpType.mult,
                )
                # y = relu(sel - 1) on the scalar engine, cast to fp16
                y_t = yp.tile([P, cols, H], F16, tag="y")
                nc.scalar.activation(y_t[:], sel_t[:],
                                     mybir.ActivationFunctionType.Relu,
                                     bias=bias_t[:])
                # ps[p,g,h] = sum_d y
                ps_t = pp.tile([P, ng, H], FP, tag="ps")
                nc.vector.tensor_reduce(
                    ps_t[:],
                    y_t[:].rearrange("p (g d) h -> p g h d", d=dd),
                    mybir.AxisListType.X,
                    mybir.AluOpType.add,
                )
                nc.sync.dma_start(
                    ps_out[:, g0 * H:(g0 + ng) * H]
                    .rearrange("p (g h) -> p g h", h=H), ps_t[:])
                nc.sync.dma_start(
                    y_out[:, col0 * H:(col0 + cols) * H]
                    .rearrange("p (c h) -> p c h", h=H), y_t[:])

    nc.compile()
    return nc


# ---------------------------------------------------------------------------
# Launch C: denominators
# ---------------------------------------------------------------------------

def _build_launch_c():
    nc = bacc.Bacc("TRN2", target_bir_lowering=False, debug=False,
                   num_devices=NCORES)
    psn = nc.dram_tensor("psn", [NSP, T * H], F16, kind="ExternalInput")
    deg = nc.dram_tensor("deg", [P, GN], FP, kind="ExternalInput")
    r_out = nc.dram_tensor("r_out", [NSP, H], FP, kind="ExternalOutput")

    with tile.TileContext(nc) as tc:
        with tc.tile_pool(name="sb", bufs=1) as sb:
            psn_t = sb.tile([P, GN, T, H], F16)
            nc.sync.dma_start(
                psn_t[:],
                psn[:].rearrange("(p q) (t h) -> p q t h", p=P, h=H))
            deg_t = sb.tile([P, GN], FP)
            nc.sync.dma_start(deg_t[:], deg[:])
            s_t = sb.tile([P, GN, H], FP)
            nc.vector.tensor_reduce(
                s_t[:],
                psn_t[:].rearrange("p q t h -> p q h t"),
                mybir.AxisListType.X,
                mybir.AluOpType.add,
            )
            nc.vector.tensor_tensor(
                s_t[:], s_t[:],
                deg_t[:].unsqueeze(2).to_broadcast([P, GN, H]),
                mybir.AluOpType.add)
            r_t = sb.tile([P, GN, H], FP)
            nc.vector.reciprocal(r_t[:], s_t[:])
            nc.sync.dma_start(
                r_out[:].rearrange("(p q) h -> p q h", p=P), r_t[:])

    nc.compile()
    return nc


# ---------------------------------------------------------------------------
# Launch D: rescale
# ---------------------------------------------------------------------------

def _build_launch_d(batches, Gv, ktot):
    nc = bacc.Bacc("TRN2", target_bir_lowering=False, debug=False,
                   num_devices=NCORES)
    y_in = nc.dram_tensor("y_in", [P, ktot * H], F16, kind="ExternalInput")
    r_vr = nc.dram_tensor("r_vr", [P, Gv * H], F16, kind="ExternalInput")
    out = nc.dram_tensor("out", [P, ktot * H], F16, kind="ExternalOutput")

    with tile.TileContext(nc) as tc:
        with (
            tc.tile_pool(name="yp", bufs=3) as yp,
            tc.tile_pool(name="rp", bufs=3) as rp,
            tc.tile_pool(name="op", bufs=3) as op,
        ):
            for (g0, ng, dd, col0) in batches:
                cols = ng * dd
                y_t = yp.tile([P, cols, H], F16, tag="y")
                nc.sync.dma_start(
                    y_t[:],
                    y_in[:, col0 * H:(col0 + cols) * H]
                    .rearrange("p (c h) -> p c h", h=H))
                r_t = rp.tile([P, ng, H], F16, tag="r")
                nc.sync.dma_start(
                    r_t[:],
                    r_vr[:, g0 * H:(g0 + ng) * H].rearrange("p (g h) -> p g h", h=H))
                rcol_t = rp.tile([P, cols, H], FP, tag="rcol")
                nc.gpsimd.tensor_copy(
                    rcol_t[:].rearrange("p (g d) h -> p g d h", d=dd),
                    r_t[:].unsqueeze(2).to_broadcast([P, ng, dd, H]))
                o_t = op.tile([P, cols, H], F16, tag="o")
                # out = (y + 1) * r
                nc.vector.scalar_tensor_tensor(
                    o_t[:], y_t[:], 1.0, rcol_t[:],
                    mybir.AluOpType.add,
                    mybir.AluOpType.mult,
                )
                nc.sync.dma_start(
                    out[:, col0 * H:(col0 + cols) * H]
                    .rearrange("p (c h) -> p c h", h=H), o_t[:])

    nc.compile()
    return nc


# ---------------------------------------------------------------------------
# Host orchestration
# ---------------------------------------------------------------------------

def kernel(feat, etype, src, dst, W_fc, edge_emb, W_e, attn_l, attn_r, attn_e):
    feat = np.asarray(feat)
    etype = np.asarray(etype).astype(np.int64)
    src = np.asarray(src).astype(np.int64)
    dst = np.asarray(dst).astype(np.int64)
    W_fc = np.asarray(W_fc)
    edge_emb = np.asarray(edge_emb)
    W_e = np.asarray(W_e)
    attn_l = np.asarray(attn_l)
    attn_r = np.asarray(attn_r)
    attn_e = np.asarray(attn_e)

    # ---------------- Launch A ----------------
    nc_a = _build_launch_a()
    attn_lr = np.concatenate(
        [attn_l.reshape(1, H * O), attn_r.reshape(1, H * O)], axis=1)
    import ml_dtypes
    in_maps_a = []
    for s in range(NCORES):
        featT_s = np.zeros((IN, NSP), ml_dtypes.bfloat16)
        featT_s[:, :NS] = feat[s * NS:(s + 1) * NS].T.astype(ml_dtypes.bfloat16)
        in_maps_a.append({
            "featT": featT_s,
            "w_fc": W_fc.astype(np.float32),
            "attn_lr": np.broadcast_to(attn_lr.astype(np.float32), (P, 2 * H * O)).copy(),
            "edge_embT": np.ascontiguousarray(edge_emb.T.astype(np.float32)),
            "w_e": W_e.astype(np.float32),
            "attn_e": np.broadcast_to(attn_e.reshape(1, H * F).astype(np.float32), (T, H * F)).copy(),
        })
    res_a = run_bass_kernel_spmd(nc_a, in_maps_a, core_ids=list(range(NCORES)))

    el_full = np.concatenate(
        [res_a.results[s]["elp"][:NS] for s in range(NCORES)])   # [N, H]
    er_full = np.concatenate(
        [res_a.results[s]["erp"][:NS] for s in range(NCORES)])   # [N, H]
    eep = res_a.results[0]["eep"]                                # [T, H]
    table = np.ascontiguousarray(el_full.reshape(NBLK, 64))

    # ---------------- host index construction (integers only) -------------
    key = dst * T + etype
    order = np.argsort(key, kind="stable")          # by dst, then etype
    dst_sorted = dst[order]
    core_bounds = np.searchsorted(dst_sorted, [c * NS for c in range(NCORES + 1)])

    # per-core vr lists (vr = (local dst, etype) with cnt >= 1), count-sorted
    pc = []
    for c in range(NCORES):
        lo, hi = core_bounds[c], core_bounds[c + 1]
        e_ids = order[lo:hi]
        keys_c = key[order[lo:hi]] - c * NS * T     # local n*T + t
        vr_keys, vr_start, vr_cnt = np.unique(
            keys_c, return_index=True, return_counts=True)
        perm = np.argsort(vr_cnt, kind="stable")    # ascending cnt
        inv = np.empty(len(perm), np.int64)
        inv[perm] = np.arange(len(perm))
        pc.append(dict(e_ids=e_ids, keys_c=keys_c, vr_keys=vr_keys,
                       vr_start=vr_start, vr_cnt=vr_cnt, perm=perm, inv=inv))

    NV = max(len(p["vr_keys"]) for p in pc)
    Gv = (NV + P - 1) // P

    # shared per-group dd (max over cores)
    dd_g = np.zeros(Gv, np.int64)
    for c in range(NCORES):
        cnt_sorted = pc[c]["vr_cnt"][pc[c]["perm"]]
        padded = np.zeros(Gv * P, np.int64)
        padded[:len(cnt_sorted)] = cnt_sorted
        dd_g = np.maximum(dd_g, padded.reshape(Gv, P).max(axis=1))

    # batches of consecutive groups padded to a uniform dd (ascending dd)
    batches = []            # (g0, n_g, dd, col0)
    colstart_g = np.zeros(Gv, np.int64)
    col0 = 0
    g = 0
    while g < Gv:
        if dd_g[g] == 0:
            g += 1
            continue
        g0 = g
        ng = 1
        ddb = int(dd_g[g])
        while (g0 + ng < Gv and dd_g[g0 + ng] > 0
               and (ng + 1) * max(ddb, int(dd_g[g0 + ng])) <= COLS_MAX):
            ddb = max(ddb, int(dd_g[g0 + ng]))
            ng += 1
        for j in range(ng):
            colstart_g[g0 + j] = col0 + j * ddb
        batches.append((g0, ng, ddb, col0))
        col0 += ng * ddb
        g = g0 + ng
    ktot = col0

    nc_b = _build_launch_b(batches, Gv, ktot)

    dd_of_g = np.zeros(Gv, np.int64)
    for (g0, ng, ddb, c0) in batches:
        dd_of_g[g0:g0 + ng] = ddb
    assert (dd_g <= dd_of_g).all(), "batch width below group degree"

    # per-core B inputs
    in_maps_b = []
    slot_edge = np.full((NCORES, P, ktot), -1, np.int64)
    vr_pos = []             # per core: (node_local, t, p, g) per vr
    for c in range(NCORES):
        d = pc[c]
        nvc = len(d["vr_keys"])
        sortpos = d["inv"]                     # vr i -> sorted position
        g_of_vr = sortpos // P
        p_of_vr = sortpos % P

        # per-edge slot assignment
        vi = np.repeat(np.arange(nvc), d["vr_cnt"])
        rank = np.arange(len(d["e_ids"])) - d["vr_start"][vi]
        pp_ = p_of_vr[vi]
        cols_ = colstart_g[g_of_vr[vi]] + rank
        srcs = src[d["e_ids"]]

        idx_grid = np.zeros((P, ktot), np.int16)
        idx_grid[pp_, cols_] = (srcs >> 3).astype(np.int16)
        kk_grid = np.full((P, ktot), -1.0, np.float32)
        kk_grid[pp_, cols_] = (srcs & 7).astype(np.float32)
        slot_edge[c, pp_, cols_] = d["e_ids"]

        # wrapped idx: per batch, position i=(j*128+p) at [i%16, i//16], x8
        idx_all = np.zeros((P, 8 * ktot), np.int16)
        for (g0, ng, ddb, c0) in batches:
            cols = ng * ddb
            flat = idx_grid[:, c0:c0 + cols].T.reshape(-1)       # i = j*128+p
            w = flat.reshape(-1, 16).T                           # [16, ni/16]
            idx_all[:, 8 * c0:8 * (c0 + cols)] = np.tile(w, (8, 1))

        # per-vr er'/ee' grids
        nodes_l = d["vr_keys"] // T
        ts = d["vr_keys"] % T
        er_g = np.zeros((P, Gv, H), np.float32)
        ee_g = np.zeros((P, Gv, H), np.float32)
        er_g[p_of_vr, g_of_vr] = er_full[c * NS + nodes_l]
        ee_g[p_of_vr, g_of_vr] = eep[ts]
        vr_pos.append((nodes_l, ts, p_of_vr, g_of_vr))

        in_maps_b.append({
            "table": table,
            "idx_all": idx_all,
            "kk_all": kk_grid.astype(np.float16),
            "iota": np.broadcast_to(
                np.arange(8, dtype=np.float16), (P, 8)).copy(),
            "er_vr": er_g.reshape(P, Gv * H).astype(np.float16),
            "ee_vr": ee_g.reshape(P, Gv * H).astype(np.float16),
        })

    res_b = run_bass_kernel_spmd(nc_b, in_maps_b, core_ids=list(range(NCORES)))

    # ---------------- Launch C ----------------
    nc_c = _build_launch_c()
    degs = np.bincount(dst, minlength=N)
    in_maps_c = []
    for c in range(NCORES):
        ps_c = res_b.results[c]["ps_out"].reshape(P, Gv, H)
        nodes_l, ts, p_v, g_v = vr_pos[c]
        psn = np.zeros((NSP, T, H), np.float32)
        psn[nodes_l, ts] = ps_c[p_v, g_v]
        deg_c = np.zeros(NSP, np.float32)
        deg_c[:NS] = degs[c * NS:(c + 1) * NS]
        deg_c = np.maximum(deg_c, 1.0)
        in_maps_c.append({
            "psn": psn.reshape(NSP, T * H).astype(np.float16),
            "deg": deg_c.reshape(P, GN),
        })
    res_c = run_bass_kernel_spmd(nc_c, in_maps_c, core_ids=list(range(NCORES)))

    # ---------------- Launch D ----------------
    nc_d = _build_launch_d(batches, Gv, ktot)
    in_maps_d = []
    for c in range(NCORES):
        r_c = res_c.results[c]["r_out"]        # [NSP, H] node-major
        nodes_l, ts, p_v, g_v = vr_pos[c]
        r_g = np.zeros((P, Gv, H), np.float32)
        r_g[p_v, g_v] = r_c[nodes_l]
        in_maps_d.append({
            "y_in": res_b.results[c]["y_out"],
            "r_vr": r_g.reshape(P, Gv * H).astype(np.float16),
        })
    res_d = run_bass_kernel_spmd(nc_d, in_maps_d, core_ids=list(range(NCORES)))

    # ---------------- unshard ----------------
    out = np.zeros((E, H), np.float32)
    for c in range(NCORES):
        o_c = res_d.results[c]["out"].reshape(P, ktot, H).astype(np.float32)
        mask = slot_edge[c] >= 0
        out[slot_edge[c][mask]] = o_c[mask]

    # timing estimate via the cost-model simulator
    try:
        from concourse.timeline_sim import TimelineSim
        _timings["A_ns"] = TimelineSim(nc_a).simulate()
        _timings["B_ns"] = TimelineSim(nc_b).simulate()
        _timings["C_ns"] = TimelineSim(nc_c).simulate()
        _timings["D_ns"] = TimelineSim(nc_d).simulate()
    except Exception as ex:  # timing must never break correctness
        _timings["error"] = repr(ex)

    return out


# revision 36
# speedup vs baseline: 4.1477x; 1.0604x over previous
"""Trainium2 Bass kernel for nn_AttentionWeight (GAT edge softmax).

out[e,h] = softmax_over_dst_segments(relu(el[src]+er[dst]+ee[etype]))

Math used on device:
  exp(relu(x)) = max(exp(x), 1)  and  exp(x) = exp(el)*exp(ee)*exp(er)
  y := exp(relu(x)) - 1 = max(el'[src]*ee'[t]*er'[dst] - 1, 0)
  segment_sum(exp(relu(x))) = sum(y) + deg   (padding slots give y = 0)
  out = (y + 1) * reciprocal(segment_sum)

Distribution (8 NeuronCores, dst-range sharded):
  Launch A: node-sharded projections. Core s owns nodes [12500s, 12500(s+1)):
    el'/er' = exp(feat @ (W_fc contracted with attn_l/attn_r)); the tiny
    edge-type table ee' = exp(contract(edge_emb@W_e, attn_e)).
  Launch B: edges grouped by virtual row vr=(dst,etype), vrs count-sorted into
    [128 x Gv] grids with per-group slot width dd. One batched dma_gather per
    column batch fetches 256B rows of the el' block table (8 nodes/row,
    idx = src>>3 fits int16); an 8-way one-hot mask select (host-built integer
    masks) picks el'[src]; multiply by ee'[t]*er'[n] per vr; y = relu(m-1) on
    the scalar engine; per-vr partial sums via strided reduce. y goes to HBM
    in fp16; partial sums in f32.
  Launch C: per-node denominators: host permutes per-vr partials to node-major
    (indexing only), C sums over etypes, adds degree, reciprocal -> r.
  Launch D: host permutes r to vr-grid order (indexing only); D streams y,
    computes (y+1)*r, writes the slot values; host scatters to edge order.

All floating-point arithmetic happens on device; the host only shards,
permutes, concatenates and builds integer index/count/mask arrays.
"""

import sys

sys.path.insert(0, "/opt/trn_rl_repo")

import numpy as np

import concourse.bass as bass
import concourse.bacc as bacc
import concourse.mybir as mybir
import concourse.tile as tile
from concourse import library_config
from concourse.bass_utils import run_bass_kernel_spmd

# problem constants (hardcoded per harness contract)
N = 100000
E = 3200000
IN = 256
H = 8
O = 64
F = 64
T = 8
NCORES = 8
P = 128

NS = N // NCORES            # 12500 nodes per shard
NSP = 12544                 # padded to 128*98
GN = NSP // P               # 98 groups of 128 nodes (launch A/C grids)
NBLK = N // 8               # 12500 el' block-table rows (8 nodes x 8 heads)
COLS_MAX = 112              # max slot columns per gather batch (ucode limit)

FP = mybir.dt.float32
F16 = mybir.dt.float16
BF16 = mybir.dt.bfloat16
I16 = mybir.dt.int16

_timings = {}


# ---------------------------------------------------------------------------
# Launch A: projections
# ---------------------------------------------------------------------------

def _build_launch_a():
    nc = bacc.Bacc("TRN2", target_bir_lowering=False, debug=False,
                   num_devices=NCORES)
    featT = nc.dram_tensor("featT", [IN, NSP], BF16, kind="ExternalInput")
    w_fc = nc.dram_tensor("w_fc", [IN, H * O], FP, kind="ExternalInput")
    attn_lr = nc.dram_tensor("attn_lr", [P, 2 * H * O], FP, kind="ExternalInput")
    edge_embT = nc.dram_tensor("edge_embT", [F, T], FP, kind="ExternalInput")
    w_e = nc.dram_tensor("w_e", [F, H * F], FP, kind="ExternalInput")
    attn_e = nc.dram_tensor("attn_e", [T, H * F], FP, kind="ExternalInput")
    elp = nc.dram_tensor("elp", [NSP, H], FP, kind="ExternalOutput")
    erp = nc.dram_tensor("erp", [NSP, H], FP, kind="ExternalOutput")
    eep = nc.dram_tensor("eep", [T, H], FP, kind="ExternalOutput")

    with tile.TileContext(nc) as tc:
        with (
            tc.tile_pool(name="sb", bufs=1) as sb,
            tc.tile_pool(name="mm", bufs=2) as mm,
            tc.tile_pool(name="ps", bufs=2, space="PSUM") as ps,
        ):
            # --- wl/wr: contract W_fc[i, h*O+o] with attn_l/r[h, o] -> [i, 2H]
            wfc_t = [sb.tile([P, H * O], FP, tag=f"wfc{c}", name=f"wfc{c}") for c in range(2)]
            for c in range(2):
                nc.sync.dma_start(wfc_t[c][:], w_fc[c * P:(c + 1) * P, :])
            alr_t = sb.tile([P, 2 * H * O], FP)
            nc.sync.dma_start(alr_t[:], attn_lr[:])
            wlr = [sb.tile([P, 2 * H], FP, tag=f"wlr{c}", name=f"wlr{c}") for c in range(2)]
            for c in range(2):
                for half in range(2):  # 0: attn_l, 1: attn_r
                    tmp = mm.tile([P, H * O], FP, tag="wtmp")
                    nc.vector.tensor_tensor(
                        tmp[:], wfc_t[c][:],
                        alr_t[:, half * H * O:(half + 1) * H * O],
                        mybir.AluOpType.mult)
                    nc.vector.tensor_reduce(
                        wlr[c][:, half * H:(half + 1) * H],
                        tmp[:].rearrange("p (h o) -> p h o", h=H),
                        mybir.AxisListType.X, mybir.AluOpType.add)

            # --- ee table: (edge_emb @ W_e) [T, H*F] contract attn_e -> [T, H]
            embT_t = sb.tile([F, T], FP)
            nc.sync.dma_start(embT_t[:], edge_embT[:])
            we_t = sb.tile([F, H * F], FP)
            nc.sync.dma_start(we_t[:], w_e[:])
            ae_t = sb.tile([T, H * F], FP)
            nc.sync.dma_start(ae_t[:], attn_e[:])
            proj_ps = ps.tile([T, H * F], FP)
            nc.tensor.matmul(proj_ps[:], lhsT=embT_t[:], rhs=we_t[:],
                             start=True, stop=True)
            proj_sb = sb.tile([T, H * F], FP)
            nc.vector.tensor_tensor(
                proj_sb[:], proj_ps[:], ae_t[:],
                mybir.AluOpType.mult)
            ee_sb = sb.tile([T, H], FP)
            nc.vector.tensor_reduce(
                ee_sb[:], proj_sb[:].rearrange("t (h f) -> t h f", h=H),
                mybir.AxisListType.X, mybir.AluOpType.add)
            eep_sb = sb.tile([T, H], FP)
            nc.scalar.activation(eep_sb[:], ee_sb[:],
                                 mybir.ActivationFunctionType.Exp)
            nc.sync.dma_start(eep[:], eep_sb[:])

            # --- el/er for the shard: node ln = p*GN + tt handled by
            #     (tile tt, psum partition p)
            wlr_bf = [sb.tile([P, 2 * H], BF16, tag=f"wlrb{c}", name=f"wlrb{c}")
                      for c in range(2)]
            for c in range(2):
                nc.vector.tensor_copy(wlr_bf[c][:], wlr[c][:])
            ftT = [sb.tile([P, NSP], BF16, tag=f"ft{c}", name=f"ft{c}") for c in range(2)]
            for c in range(2):
                nc.sync.dma_start(ftT[c][:], featT[c * P:(c + 1) * P, :])
            elr = sb.tile([P, GN, 2 * H], FP)
            SLICES = 32
            tt = 0
            while tt < GN:
                nsl = min(SLICES, GN - tt)
                bank = ps.tile([P, SLICES * 2 * H], FP, tag="bank")
                for j in range(nsl):
                    sl = bank[:, j * 2 * H:(j + 1) * 2 * H]
                    for c in range(2):
                        lhsT = ftT[c][:].rearrange("i (p t) -> i t p", p=P)[:, tt + j, :]
                        nc.tensor.matmul(sl, lhsT=lhsT, rhs=wlr_bf[c][:],
                                         start=(c == 0), stop=(c == 1))
                nc.scalar.activation(
                    elr[:, tt:tt + nsl, :],
                    bank[:, :nsl * 2 * H].rearrange("p (t h) -> p t h", h=2 * H),
                    mybir.ActivationFunctionType.Exp)
                tt += nsl
            # write out: partition p holds nodes [GN*p, GN*(p+1))
            nc.sync.dma_start(
                elp[:].rearrange("(p t) h -> p t h", p=P), elr[:, :, 0:H])
            nc.sync.dma_start(
                erp[:].rearrange("(p t) h -> p t h", p=P), elr[:, :, H:2 * H])

    nc.compile()
    return nc


# ---------------------------------------------------------------------------
# Launch B: gather + numerators + per-vr partial sums
# ---------------------------------------------------------------------------

def _build_launch_b(batches, Gv, ktot):
    """batches: list of (g0, n_g, dd, col0); Gv groups, ktot slot columns."""
    nc = bacc.Bacc("TRN2", target_bir_lowering=False, debug=False,
                   num_devices=NCORES, dynamic_dma_scratch_size=32768)
    table = nc.dram_tensor("table", [NBLK, 64], FP, kind="ExternalInput")
    idx_all = nc.dram_tensor("idx_all", [P, 8 * ktot], I16, kind="ExternalInput")
    kk_all = nc.dram_tensor("kk_all", [P, ktot], F16, kind="ExternalInput")
    iota = nc.dram_tensor("iota", [P, 8], F16, kind="ExternalInput")
    er_vr = nc.dram_tensor("er_vr", [P, Gv * H], F16, kind="ExternalInput")
    ee_vr = nc.dram_tensor("ee_vr", [P, Gv * H], F16, kind="ExternalInput")
    ps_out = nc.dram_tensor("ps_out", [P, Gv * H], FP, kind="ExternalOutput")
    y_out = nc.dram_tensor("y_out", [P, ktot * H], F16, kind="ExternalOutput")

    with tile.TileContext(nc) as tc:
        with (
            tc.tile_pool(name="cst", bufs=1) as cst,
            tc.tile_pool(name="gp", bufs=3) as gp,
            tc.tile_pool(name="ip", bufs=3) as ip,
            tc.tile_pool(name="tp", bufs=3) as tp,
            tc.tile_pool(name="sp", bufs=3) as sp,
            tc.tile_pool(name="yp", bufs=3) as yp,
            tc.tile_pool(name="pp", bufs=3) as pp,
        ):
            nc.gpsimd.load_library(library_config.mlp)
            bias_t = cst.tile([P, 1], FP)
            nc.vector.memset(bias_t[:], -1.0)
            iota_t = cst.tile([P, 8], F16)
            nc.sync.dma_start(iota_t[:], iota[:])

            for (g0, ng, dd, col0) in batches:
                cols = ng * dd
                ni = P * cols
                idx_t = ip.tile([P, 8 * cols], I16, tag="idx")
                nc.sync.dma_start(idx_t[:], idx_all[:, 8 * col0:8 * (col0 + cols)])
                kk_t = ip.tile([P, cols], F16, tag="kk")
                nc.sync.dma_start(kk_t[:], kk_all[:, col0:col0 + cols])
                erv_t = ip.tile([P, ng, H], F16, tag="erv")
                nc.sync.dma_start(
                    erv_t[:],
                    er_vr[:, g0 * H:(g0 + ng) * H].rearrange("p (g h) -> p g h", h=H))
                eev_t = ip.tile([P, ng, H], F16, tag="eev")
                nc.sync.dma_start(
                    eev_t[:],
                    ee_vr[:, g0 * H:(g0 + ng) * H].rearrange("p (g h) -> p g h", h=H))

                g_t = gp.tile([P, cols, 64], FP, tag="g")
                nc.gpsimd.dma_gather(g_t[:], table[:], idx_t[:], ni, ni, 64,
                                     single_packet=False)

                # vm = er' * ee' per vr
                vm_t = sp.tile([P, ng, H], FP, tag="vm")
                nc.vector.tensor_tensor(vm_t[:], erv_t[:], eev_t[:],
                                        mybir.AluOpType.mult)
                # one-hot(k) from kk vs iota (pad slots have kk = -1)
                mk_t = tp.tile([P, cols, 8], FP, tag="mk")
                nc.vector.tensor_tensor(
                    mk_t[:],
                    kk_t[:].unsqueeze(2).to_broadcast([P, cols, 8]),
                    iota_t[:].unsqueeze(1).to_broadcast([P, cols, 8]),
                    mybir.AluOpType.is_equal)
                # g <- g * one-hot(k) in place (mask broadcast over h)
                nc.vector.scalar_tensor_tensor(
                    g_t[:].rearrange("p c (k h) -> p c k h", h=H),
                    g_t[:].rearrange("p c (k h) -> p c k h", h=H),
                    1.0,
                    mk_t[:].unsqueeze(3).to_broadcast([P, cols, 8, H]),
                    mybir.AluOpType.mult,
                    mybir.AluOpType.mult,
                )
                # sel[p,c,h] = sum_k g
                sel_t = sp.tile([P, cols, H], FP, tag="sel")
                nc.vector.tensor_reduce(
                    sel_t[:],
                    g_t[:].rearrange("p c (k h) -> p c h k", h=H),
                    mybir.AxisListType.X,
                    mybir.AluOpType.add,
                )
                # sel <- sel * vm in place (vm broadcast over dd)
                nc.vector.tensor_tensor(
                    sel_t[:].rearrange("p (g d) h -> p g d h", d=dd),
                    sel_t[:].rearrange("p (g d) h -> p g d h", d=dd),
                    vm_t[:].unsqueeze(2).to_broadcast([P, ng, dd, H]),
                    mybir.AluOpType.mult,
                )
                # y = relu(sel - 1) on the scalar engine, cast to fp16
                y_t = yp.tile([P, cols, H], F16, tag="y")
                nc.scalar.activation(y_t[:], sel_t[:],
                                     mybir.ActivationFunctionType.Relu,
                                     bias=bias_t[:])
                # ps[p,g,h] = sum_d y
                ps_t = pp.tile([P, ng, H], FP, tag="ps")
                nc.vector.tensor_reduce(
                    ps_t[:],
                    y_t[:].rearrange("p (g d) h -> p g h d", d=dd),
                    mybir.AxisListType.X,
                    mybir.AluOpType.add,
                )
                nc.sync.dma_start(
                    ps_out[:, g0 * H:(g0 + ng) * H]
                    .rearrange("p (g h) -> p g h", h=H), ps_t[:])
                nc.sync.dma_start(
                    y_out[:, col0 * H:(col0 + cols) * H]
                    .rearrange("p (c h) -> p c h", h=H), y_t[:])

    nc.compile()
    return nc


# ---------------------------------------------------------------------------
# Launch C: denominators
# ---------------------------------------------------------------------------

def _build_launch_c():
    nc = bacc.Bacc("TRN2", target_bir_lowering=False, debug=False,
                   num_devices=NCORES)
    psn = nc.dram_tensor("psn", [NSP, T * H], F16, kind="ExternalInput")
    deg = nc.dram_tensor("deg", [P, GN], FP, kind="ExternalInput")
    r_out = nc.dram_tensor("r_out", [NSP, H], FP, kind="ExternalOutput")

    with tile.TileContext(nc) as tc:
        with tc.tile_pool(name="sb", bufs=1) as sb:
            psn_t = sb.tile([P, GN, T, H], F16)
            nc.sync.dma_start(
                psn_t[:],
                psn[:].rearrange("(p q) (t h) -> p q t h", p=P, h=H))
            deg_t = sb.tile([P, GN], FP)
            nc.sync.dma_start(deg_t[:], deg[:])
            s_t = sb.tile([P, GN, H], FP)
            nc.vector.tensor_reduce(
                s_t[:],
                psn_t[:].rearrange("p q t h -> p q h t"),
                mybir.AxisListType.X,
                mybir.AluOpType.add,
            )
            nc.vector.tensor_tensor(
                s_t[:], s_t[:],
                deg_t[:].unsqueeze(2).to_broadcast([P, GN, H]),
                mybir.AluOpType.add)
            r_t = sb.tile([P, GN, H], FP)
            nc.vector.reciprocal(r_t[:], s_t[:])
            nc.sync.dma_start(
                r_out[:].rearrange("(p q) h -> p q h", p=P), r_t[:])

    nc.compile()
    return nc


# ---------------------------------------------------------------------------
# Launch D: rescale
# ---------------------------------------------------------------------------

def _build_launch_d(batches, Gv, ktot):
    nc = bacc.Bacc("TRN2", target_bir_lowering=False, debug=False,
                   num_devices=NCORES)
    y_in = nc.dram_tensor("y_in", [P, ktot * H], F16, kind="ExternalInput")
    r_vr = nc.dram_tensor("r_vr", [P, Gv * H], F16, kind="ExternalInput")
    out = nc.dram_tensor("out", [P, ktot * H], F16, kind="ExternalOutput")

    with tile.TileContext(nc) as tc:
        with (
            tc.tile_pool(name="yp", bufs=3) as yp,
            tc.tile_pool(name="rp", bufs=3) as rp,
            tc.tile_pool(name="op", bufs=3) as op,
        ):
            for (g0, ng, dd, col0) in batches:
                cols = ng * dd
                y_t = yp.tile([P, cols, H], F16, tag="y")
                nc.sync.dma_start(
                    y_t[:],
                    y_in[:, col0 * H:(col0 + cols) * H]
                    .rearrange("p (c h) -> p c h", h=H))
                r_t = rp.tile([P, ng, H], F16, tag="r")
                nc.sync.dma_start(
                    r_t[:],
                    r_vr[:, g0 * H:(g0 + ng) * H].rearrange("p (g h) -> p g h", h=H))
                rcol_t = rp.tile([P, cols, H], FP, tag="rcol")
                nc.gpsimd.tensor_copy(
                    rcol_t[:].rearrange("p (g d) h -> p g d h", d=dd),
                    r_t[:].unsqueeze(2).to_broadcast([P, ng, dd, H]))
                o_t = op.tile([P, cols, H], F16, tag="o")
                # out = (y + 1) * r
                nc.vector.scalar_tensor_tensor(
                    o_t[:], y_t[:], 1.0, rcol_t[:],
                    mybir.AluOpType.add,
                    mybir.AluOpType.mult,
                )
                nc.sync.dma_start(
                    out[:, col0 * H:(col0 + cols) * H]
                    .rearrange("p (c h) -> p c h", h=H), o_t[:])

    nc.compile()
    return nc


# ---------------------------------------------------------------------------
# Host orchestration
# ---------------------------------------------------------------------------

def kernel(feat, etype, src, dst, W_fc, edge_emb, W_e, attn_l, attn_r, attn_e):
    feat = np.asarray(feat)
    etype = np.asarray(etype).astype(np.int64)
    src = np.asarray(src).astype(np.int64)
    dst = np.asarray(dst).astype(np.int64)
    W_fc = np.asarray(W_fc)
    edge_emb = np.asarray(edge_emb)
    W_e = np.asarray(W_e)
    attn_l = np.asarray(attn_l)
    attn_r = np.asarray(attn_r)
    attn_e = np.asarray(attn_e)

    # ---------------- Launch A ----------------
    nc_a = _build_launch_a()
    attn_lr = np.concatenate(
        [attn_l.reshape(1, H * O), attn_r.reshape(1, H * O)], axis=1)
    import ml_dtypes
    in_maps_a = []
    for s in range(NCORES):
        featT_s = np.zeros((IN, NSP), ml_dtypes.bfloat16)
        featT_s[:, :NS] = feat[s * NS:(s + 1) * NS].T.astype(ml_dtypes.bfloat16)
        in_maps_a.append({
            "featT": featT_s,
            "w_fc": W_fc.astype(np.float32),
            "attn_lr": np.broadcast_to(attn_lr.astype(np.float32), (P, 2 * H * O)).copy(),
            "edge_embT": np.ascontiguousarray(edge_emb.T.astype(np.float32)),
            "w_e": W_e.astype(np.float32),
            "attn_e": np.broadcast_to(attn_e.reshape(1, H * F).astype(np.float32), (T, H * F)).copy(),
        })
    res_a = run_bass_kernel_spmd(nc_a, in_maps_a, core_ids=list(range(NCORES)))

    el_full = np.concatenate(
        [res_a.results[s]["elp"][:NS] for s in range(NCORES)])   # [N, H]
    er_full = np.concatenate(
        [res_a.results[s]["erp"][:NS] for s in range(NCORES)])   # [N, H]
    eep = res_a.results[0]["eep"]                                # [T, H]
    table = np.ascontiguousarray(el_full.reshape(NBLK, 64))

    # ---------------- host index construction (integers only) -------------
    key = dst * T + etype
    order = np.argsort(key, kind="stable")          # by dst, then etype
    dst_sorted = dst[order]
    core_bounds = np.searchsorted(dst_sorted, [c * NS for c in range(NCORES + 1)])

    # per-core vr lists (vr = (local dst, etype) with cnt >= 1), count-sorted
    pc = []
    for c in range(NCORES):
        lo, hi = core_bounds[c], core_bounds[c + 1]
        e_ids = order[lo:hi]
        keys_c = key[order[lo:hi]] - c * NS * T     # local n*T + t
        vr_keys, vr_start, vr_cnt = np.unique(
            keys_c, return_index=True, return_counts=True)
        perm = np.argsort(vr_cnt, kind="stable")    # ascending cnt
        inv = np.empty(len(perm), np.int64)
        inv[perm] = np.arange(len(perm))
        pc.append(dict(e_ids=e_ids, keys_c=keys_c, vr_keys=vr_keys,
                       vr_start=vr_start, vr_cnt=vr_cnt, perm=perm, inv=inv))

    NV = max(len(p["vr_keys"]) for p in pc)
    Gv = (NV + P - 1) // P

    # shared per-group dd (max over cores)
    dd_g = np.zeros(Gv, np.int64)
    for c in range(NCORES):
        cnt_sorted = pc[c]["vr_cnt"][pc[c]["perm"]]
        padded = np.zeros(Gv * P, np.int64)
        padded[:len(cnt_sorted)] = cnt_sorted
        dd_g = np.maximum(dd_g, padded.reshape(Gv, P).max(axis=1))

    # batches of consecutive groups padded to a uniform dd (ascending dd)
    batches = []            # (g0, n_g, dd, col0)
    colstart_g = np.zeros(Gv, np.int64)
    col0 = 0
    g = 0
    while g < Gv:
        if dd_g[g] == 0:
            g += 1
            continue
        g0 = g
        ng = 1
        ddb = int(dd_g[g])
        while (g0 + ng < Gv and dd_g[g0 + ng] > 0
               and (ng + 1) * max(ddb, int(dd_g[g0 + ng])) <= COLS_MAX):
            ddb = max(ddb, int(dd_g[g0 + ng]))
            ng += 1
        for j in range(ng):
            colstart_g[g0 + j] = col0 + j * ddb
        batches.append((g0, ng, ddb, col0))
        col0 += ng * ddb
        g = g0 + ng
    ktot = col0

    nc_b = _build_launch_b(batches, Gv, ktot)

    dd_of_g = np.zeros(Gv, np.int64)
    for (g0, ng, ddb, c0) in batches:
        dd_of_g[g0:g0 + ng] = ddb
    assert (dd_g <= dd_of_g).all(), "batch width below group degree"

    # per-core B inputs
    in_maps_b = []
    slot_edge = np.full((NCORES, P, ktot), -1, np.int64)
    vr_pos = []             # per core: (node_local, t, p, g) per vr
    for c in range(NCORES):
        d = pc[c]
        nvc = len(d["vr_keys"])
        sortpos = d["inv"]                     # vr i -> sorted position
        g_of_vr = sortpos // P
        p_of_vr = sortpos % P

        # per-edge slot assignment
        vi = np.repeat(np.arange(nvc), d["vr_cnt"])
        rank = np.arange(len(d["e_ids"])) - d["vr_start"][vi]
        pp_ = p_of_vr[vi]
        cols_ = colstart_g[g_of_vr[vi]] + rank
        srcs = src[d["e_ids"]]

        idx_grid = np.zeros((P, ktot), np.int16)
        idx_grid[pp_, cols_] = (srcs >> 3).astype(np.int16)
        kk_grid = np.full((P, ktot), -1.0, np.float32)
        kk_grid[pp_, cols_] = (srcs & 7).astype(np.float32)
        slot_edge[c, pp_, cols_] = d["e_ids"]

        # wrapped idx: per batch, position i=(j*128+p) at [i%16, i//16], x8
        idx_all = np.zeros((P, 8 * ktot), np.int16)
        for (g0, ng, ddb, c0) in batches:
            cols = ng * ddb
            flat = idx_grid[:, c0:c0 + cols].T.reshape(-1)       # i = j*128+p
            w = flat.reshape(-1, 16).T                           # [16, ni/16]
            idx_all[:, 8 * c0:8 * (c0 + cols)] = np.tile(w, (8, 1))

        # per-vr er'/ee' grids
        nodes_l = d["vr_keys"] // T
        ts = d["vr_keys"] % T
        er_g = np.zeros((P, Gv, H), np.float32)
        ee_g = np.zeros((P, Gv, H), np.float32)
        er_g[p_of_vr, g_of_vr] = er_full[c * NS + nodes_l]
        ee_g[p_of_vr, g_of_vr] = eep[ts]
        vr_pos.append((nodes_l, ts, p_of_vr, g_of_vr))

        in_maps_b.append({
            "table": table,
            "idx_all": idx_all,
            "kk_all": kk_grid.astype(np.float16),
            "iota": np.broadcast_to(
                np.arange(8, dtype=np.float16), (P, 8)).copy(),
            "er_vr": er_g.reshape(P, Gv * H).astype(np.float16),
            "ee_vr": ee_g.reshape(P, Gv * H).astype(np.float16),
        })

    res_b = run_bass_kernel_spmd(nc_b, in_maps_b, core_ids=list(range(NCORES)))

    # ---------------- Launch C ----------------
    nc_c = _build_launch_c()
    degs = np.bincount(dst, minlength=N)
    in_maps_c = []
    for c in range(NCORES):
        ps_c = res_b.results[c]["ps_out"].reshape(P, Gv, H)
        nodes_l, ts, p_v, g_v = vr_pos[c]
        psn = np.zeros((NSP, T, H), np.float32)
        psn[nodes_l, ts] = ps_c[p_v, g_v]
        deg_c = np.zeros(NSP, np.float32)
        deg_c[:NS] = degs[c * NS:(c + 1) * NS]
        deg_c = np.maximum(deg_c, 1.0)
        in_maps_c.append({
            "psn": psn.reshape(NSP, T * H).astype(np.float16),
            "deg": deg_c.reshape(P, GN),
        })
    res_c = run_bass_kernel_spmd(nc_c, in_maps_c, core_ids=list(range(NCORES)))

    # ---------------- Launch D ----------------
    nc_d = _build_launch_d(batches, Gv, ktot)
    in_maps_d = []
    for c in range(NCORES):
        r_c = res_c.results[c]["r_out"]        # [NSP, H] node-major
        nodes_l, ts, p_v, g_v = vr_pos[c]
        r_g = np.zeros((P, Gv, H), np.float32)
        r_g[p_v, g_v] = r_c[nodes_l]
        in_maps_d.append({
            "y_in": res_b.results[c]["y_out"],
            "r_vr": r_g.reshape(P, Gv * H).astype(np.float16),
        })
    res_d = run_bass_kernel_spmd(nc_d, in_maps_d, core_ids=list(range(NCORES)))

    # ---------------- unshard ----------------
    out = np.zeros((E, H), np.float32)
    for c in range(NCORES):
        o_c = res_d.results[c]["out"].reshape(P, ktot, H).astype(np.float32)
        mask = slot_edge[c] >= 0
        out[slot_edge[c][mask]] = o_c[mask]

    # timing estimate via the cost-model simulator
    try:
        from concourse.timeline_sim import TimelineSim
        _timings["A_ns"] = TimelineSim(nc_a).simulate()
        _timings["B_ns"] = TimelineSim(nc_b).simulate()
        _timings["C_ns"] = TimelineSim(nc_c).simulate()
        _timings["D_ns"] = TimelineSim(nc_d).simulate()
    except Exception as ex:  # timing must never break correctness
        _timings["error"] = repr(ex)

    return out


# revision 39
# speedup vs baseline: 4.1558x; 1.0020x over previous
"""Trainium2 Bass kernel for nn_AttentionWeight (GAT edge softmax).

out[e,h] = softmax_over_dst_segments(relu(el[src]+er[dst]+ee[etype]))

Math used on device:
  exp(relu(x)) = max(exp(x), 1)  and  exp(x) = exp(el)*exp(ee)*exp(er)
  y := exp(relu(x)) - 1 = max(el'[src]*ee'[t]*er'[dst] - 1, 0)
  segment_sum(exp(relu(x))) = sum(y) + deg   (padding slots give y = 0)
  out = (y + 1) * reciprocal(segment_sum)

Distribution (8 NeuronCores, dst-range sharded):
  Launch A: node-sharded projections. Core s owns nodes [12500s, 12500(s+1)):
    el'/er' = exp(feat @ (W_fc contracted with attn_l/attn_r)) (feat in bf16);
    the tiny edge-type table ee' = exp(contract(edge_emb@W_e, attn_e)).
  Launch B: edges grouped by virtual row vr=(dst,etype), vrs count-sorted into
    [128 x Gv] grids with per-group slot width dd (uniform per batch). One
    batched dma_gather per column batch (<=14336 indices, the SWDGE ucode
    limit) fetches 256B rows of the el' block table (8 nodes/row,
    idx = src>>3 fits the gather's int16 index); the 8-way sub-block select
    builds a one-hot on device (is_equal of a host-provided k grid vs iota)
    and mask-multiplies the gathered rows in place, then a strided reduce
    picks el'[src]; multiply by ee'[t]*er'[n] per vr; y = relu(m-1) on the
    scalar engine (fp16 out); per-vr partial sums via strided reduce (f32).
    This replaces the previous per-column indirect-DMA gather whose ~1us
    SWDGE fixed cost per instruction dominated the kernel; the batched
    gather's cost is ~0.34ns/descriptor on Pool plus the 256B-descriptor DMA
    transfer, which makes launch B ~90% DMA-engine-bound.
  Launch C: per-node denominators: host permutes per-vr partials to node-major
    (indexing only), C sums over etypes, adds degree, reciprocal -> r.
  Launch D: host replicates r to per-slot-column order (indexing only); D
    streams y, computes (y+1)*r, writes fp16 slot values; host scatters them
    back to edge order.

All floating-point arithmetic happens on device; the host only shards,
permutes, replicates, concatenates, casts dtypes and builds integer
index/count arrays.
"""

import sys

sys.path.insert(0, "/opt/trn_rl_repo")

import numpy as np

import concourse.bass as bass
import concourse.bacc as bacc
import concourse.mybir as mybir
import concourse.tile as tile
from concourse import library_config
from concourse.bass_utils import run_bass_kernel_spmd

# problem constants (hardcoded per harness contract)
N = 100000
E = 3200000
IN = 256
H = 8
O = 64
F = 64
T = 8
NCORES = 8
P = 128

NS = N // NCORES            # 12500 nodes per shard
NSP = 12544                 # padded to 128*98
GN = NSP // P               # 98 groups of 128 nodes (launch A/C grids)
NBLK = N // 8               # 12500 el' block-table rows (8 nodes x 8 heads)
COLS_MAX = 112              # max slot columns per gather batch (ucode limit)

FP = mybir.dt.float32
F16 = mybir.dt.float16
BF16 = mybir.dt.bfloat16
I16 = mybir.dt.int16

_timings = {}


# ---------------------------------------------------------------------------
# Launch A: projections
# ---------------------------------------------------------------------------

def _build_launch_a():
    nc = bacc.Bacc("TRN2", target_bir_lowering=False, debug=False,
                   num_devices=NCORES)
    featT = nc.dram_tensor("featT", [IN, NSP], BF16, kind="ExternalInput")
    w_fc = nc.dram_tensor("w_fc", [IN, H * O], FP, kind="ExternalInput")
    attn_lr = nc.dram_tensor("attn_lr", [P, 2 * H * O], FP, kind="ExternalInput")
    edge_embT = nc.dram_tensor("edge_embT", [F, T], FP, kind="ExternalInput")
    w_e = nc.dram_tensor("w_e", [F, H * F], FP, kind="ExternalInput")
    attn_e = nc.dram_tensor("attn_e", [T, H * F], FP, kind="ExternalInput")
    elp = nc.dram_tensor("elp", [NSP, H], FP, kind="ExternalOutput")
    erp = nc.dram_tensor("erp", [NSP, H], FP, kind="ExternalOutput")
    eep = nc.dram_tensor("eep", [T, H], FP, kind="ExternalOutput")

    with tile.TileContext(nc) as tc:
        with (
            tc.tile_pool(name="sb", bufs=1) as sb,
            tc.tile_pool(name="mm", bufs=2) as mm,
            tc.tile_pool(name="ps", bufs=2, space="PSUM") as ps,
        ):
            # --- wl/wr: contract W_fc[i, h*O+o] with attn_l/r[h, o] -> [i, 2H]
            wfc_t = [sb.tile([P, H * O], FP, tag=f"wfc{c}", name=f"wfc{c}") for c in range(2)]
            for c in range(2):
                nc.sync.dma_start(wfc_t[c][:], w_fc[c * P:(c + 1) * P, :])
            alr_t = sb.tile([P, 2 * H * O], FP)
            nc.sync.dma_start(alr_t[:], attn_lr[:])
            wlr = [sb.tile([P, 2 * H], FP, tag=f"wlr{c}", name=f"wlr{c}") for c in range(2)]
            for c in range(2):
                for half in range(2):  # 0: attn_l, 1: attn_r
                    tmp = mm.tile([P, H * O], FP, tag="wtmp")
                    nc.vector.tensor_tensor(
                        tmp[:], wfc_t[c][:],
                        alr_t[:, half * H * O:(half + 1) * H * O],
                        mybir.AluOpType.mult)
                    nc.vector.tensor_reduce(
                        wlr[c][:, half * H:(half + 1) * H],
                        tmp[:].rearrange("p (h o) -> p h o", h=H),
                        mybir.AxisListType.X, mybir.AluOpType.add)

            # --- ee table: (edge_emb @ W_e) [T, H*F] contract attn_e -> [T, H]
            embT_t = sb.tile([F, T], FP)
            nc.sync.dma_start(embT_t[:], edge_embT[:])
            we_t = sb.tile([F, H * F], FP)
            nc.sync.dma_start(we_t[:], w_e[:])
            ae_t = sb.tile([T, H * F], FP)
            nc.sync.dma_start(ae_t[:], attn_e[:])
            proj_ps = ps.tile([T, H * F], FP)
            nc.tensor.matmul(proj_ps[:], lhsT=embT_t[:], rhs=we_t[:],
                             start=True, stop=True)
            proj_sb = sb.tile([T, H * F], FP)
            nc.vector.tensor_tensor(
                proj_sb[:], proj_ps[:], ae_t[:],
                mybir.AluOpType.mult)
            ee_sb = sb.tile([T, H], FP)
            nc.vector.tensor_reduce(
                ee_sb[:], proj_sb[:].rearrange("t (h f) -> t h f", h=H),
                mybir.AxisListType.X, mybir.AluOpType.add)
            eep_sb = sb.tile([T, H], FP)
            nc.scalar.activation(eep_sb[:], ee_sb[:],
                                 mybir.ActivationFunctionType.Exp)
            nc.sync.dma_start(eep[:], eep_sb[:])

            # --- el/er for the shard: node ln = p*GN + tt handled by
            #     (tile tt, psum partition p)
            wlr_bf = [sb.tile([P, 2 * H], BF16, tag=f"wlrb{c}", name=f"wlrb{c}")
                      for c in range(2)]
            for c in range(2):
                nc.vector.tensor_copy(wlr_bf[c][:], wlr[c][:])
            ftT = [sb.tile([P, NSP], BF16, tag=f"ft{c}", name=f"ft{c}") for c in range(2)]
            for c in range(2):
                nc.sync.dma_start(ftT[c][:], featT[c * P:(c + 1) * P, :])
            elr = sb.tile([P, GN, 2 * H], FP)
            SLICES = 32
            tt = 0
            while tt < GN:
                nsl = min(SLICES, GN - tt)
                bank = ps.tile([P, SLICES * 2 * H], FP, tag="bank")
                for j in range(nsl):
                    sl = bank[:, j * 2 * H:(j + 1) * 2 * H]
                    for c in range(2):
                        lhsT = ftT[c][:].rearrange("i (p t) -> i t p", p=P)[:, tt + j, :]
                        nc.tensor.matmul(sl, lhsT=lhsT, rhs=wlr_bf[c][:],
                                         start=(c == 0), stop=(c == 1))
                nc.scalar.activation(
                    elr[:, tt:tt + nsl, :],
                    bank[:, :nsl * 2 * H].rearrange("p (t h) -> p t h", h=2 * H),
                    mybir.ActivationFunctionType.Exp)
                tt += nsl
            # write out: partition p holds nodes [GN*p, GN*(p+1))
            nc.sync.dma_start(
                elp[:].rearrange("(p t) h -> p t h", p=P), elr[:, :, 0:H])
            nc.sync.dma_start(
                erp[:].rearrange("(p t) h -> p t h", p=P), elr[:, :, H:2 * H])

    nc.compile()
    return nc


# ---------------------------------------------------------------------------
# Launch B: gather + numerators + per-vr partial sums
# ---------------------------------------------------------------------------

def _build_launch_b(batches, Gv, ktot):
    """batches: list of (g0, n_g, dd, col0); Gv groups, ktot slot columns."""
    nc = bacc.Bacc("TRN2", target_bir_lowering=False, debug=False,
                   num_devices=NCORES, dynamic_dma_scratch_size=32768)
    table = nc.dram_tensor("table", [NBLK, 64], FP, kind="ExternalInput")
    idx_all = nc.dram_tensor("idx_all", [P, 8 * ktot], I16, kind="ExternalInput")
    kk_all = nc.dram_tensor("kk_all", [P, ktot], F16, kind="ExternalInput")
    iota = nc.dram_tensor("iota", [P, 8], F16, kind="ExternalInput")
    er_vr = nc.dram_tensor("er_vr", [P, Gv * H], F16, kind="ExternalInput")
    ee_vr = nc.dram_tensor("ee_vr", [P, Gv * H], F16, kind="ExternalInput")
    ps_out = nc.dram_tensor("ps_out", [P, Gv * H], FP, kind="ExternalOutput")
    y_out = nc.dram_tensor("y_out", [P, ktot * H], F16, kind="ExternalOutput")

    with tile.TileContext(nc) as tc:
        with (
            tc.tile_pool(name="cst", bufs=1) as cst,
            tc.tile_pool(name="gp", bufs=3) as gp,
            tc.tile_pool(name="ip", bufs=3) as ip,
            tc.tile_pool(name="tp", bufs=3) as tp,
            tc.tile_pool(name="sp", bufs=3) as sp,
            tc.tile_pool(name="yp", bufs=3) as yp,
            tc.tile_pool(name="pp", bufs=3) as pp,
        ):
            nc.gpsimd.load_library(library_config.mlp)
            bias_t = cst.tile([P, 1], FP)
            nc.vector.memset(bias_t[:], -1.0)
            iota_t = cst.tile([P, 8], F16)
            nc.sync.dma_start(iota_t[:], iota[:])

            for (g0, ng, dd, col0) in batches:
                cols = ng * dd
                ni = P * cols
                idx_t = ip.tile([P, 8 * cols], I16, tag="idx")
                nc.sync.dma_start(idx_t[:], idx_all[:, 8 * col0:8 * (col0 + cols)])
                kk_t = ip.tile([P, cols], F16, tag="kk")
                nc.sync.dma_start(kk_t[:], kk_all[:, col0:col0 + cols])
                erv_t = ip.tile([P, ng, H], F16, tag="erv")
                nc.sync.dma_start(
                    erv_t[:],
                    er_vr[:, g0 * H:(g0 + ng) * H].rearrange("p (g h) -> p g h", h=H))
                eev_t = ip.tile([P, ng, H], F16, tag="eev")
                nc.sync.dma_start(
                    eev_t[:],
                    ee_vr[:, g0 * H:(g0 + ng) * H].rearrange("p (g h) -> p g h", h=H))

                g_t = gp.tile([P, cols, 64], FP, tag="g")
                nc.gpsimd.dma_gather(g_t[:], table[:], idx_t[:], ni, ni, 64,
                                     single_packet=False)

                # vm = er' * ee' per vr
                vm_t = sp.tile([P, ng, H], FP, tag="vm")
                nc.vector.tensor_tensor(vm_t[:], erv_t[:], eev_t[:],
                                        mybir.AluOpType.mult)
                # one-hot(k) from kk vs iota (pad slots have kk = -1)
                mk_t = tp.tile([P, cols, 8], FP, tag="mk")
                nc.vector.tensor_tensor(
                    mk_t[:],
                    kk_t[:].unsqueeze(2).to_broadcast([P, cols, 8]),
                    iota_t[:].unsqueeze(1).to_broadcast([P, cols, 8]),
                    mybir.AluOpType.is_equal)
                # g <- g * one-hot(k) in place (mask broadcast over h)
                nc.vector.scalar_tensor_tensor(
                    g_t[:].rearrange("p c (k h) -> p c k h", h=H),
                    g_t[:].rearrange("p c (k h) -> p c k h", h=H),
                    1.0,
                    mk_t[:].unsqueeze(3).to_broadcast([P, cols, 8, H]),
                    mybir.AluOpType.mult,
                    mybir.AluOpType.mult,
                )
                # sel[p,c,h] = sum_k g
                sel_t = sp.tile([P, cols, H], FP, tag="sel")
                nc.vector.tensor_reduce(
                    sel_t[:],
                    g_t[:].rearrange("p c (k h) -> p c h k", h=H),
                    mybir.AxisListType.X,
                    mybir.AluOpType.add,
                )
                # sel <- sel * vm in place (vm broadcast over dd)
                nc.vector.tensor_tensor(
                    sel_t[:].rearrange("p (g d) h -> p g d h", d=dd),
                    sel_t[:].rearrange("p (g d) h -> p g d h", d=dd),
                    vm_t[:].unsqueeze(2).to_broadcast([P, ng, dd, H]),
                    mybir.AluOpType.mult,
                )
                # y = relu(sel - 1) on the scalar engine, cast to fp16
                y_t = yp.tile([P, cols, H], F16, tag="y")
                nc.scalar.activation(y_t[:], sel_t[:],
                                     mybir.ActivationFunctionType.Relu,
                                     bias=bias_t[:])
                # ps[p,g,h] = sum_d y
                ps_t = pp.tile([P, ng, H], FP, tag="ps")
                nc.vector.tensor_reduce(
                    ps_t[:],
                    y_t[:].rearrange("p (g d) h -> p g h d", d=dd),
                    mybir.AxisListType.X,
                    mybir.AluOpType.add,
                )
                nc.sync.dma_start(
                    ps_out[:, g0 * H:(g0 + ng) * H]
                    .rearrange("p (g h) -> p g h", h=H), ps_t[:])
                nc.sync.dma_start(
                    y_out[:, col0 * H:(col0 + cols) * H]
                    .rearrange("p (c h) -> p c h", h=H), y_t[:])

    nc.compile()
    return nc


# ---------------------------------------------------------------------------
# Launch C: denominators
# ---------------------------------------------------------------------------

def _build_launch_c():
    nc = bacc.Bacc("TRN2", target_bir_lowering=False, debug=False,
                   num_devices=NCORES)
    psn = nc.dram_tensor("psn", [NSP, T * H], F16, kind="ExternalInput")
    deg = nc.dram_tensor("deg", [P, GN], FP, kind="ExternalInput")
    r_out = nc.dram_tensor("r_out", [NSP, H], FP, kind="ExternalOutput")

    with tile.TileContext(nc) as tc:
        with tc.tile_pool(name="sb", bufs=1) as sb:
            psn_t = sb.tile([P, GN, T, H], F16)
            nc.sync.dma_start(
                psn_t[:],
                psn[:].rearrange("(p q) (t h) -> p q t h", p=P, h=H))
            deg_t = sb.tile([P, GN], FP)
            nc.sync.dma_start(deg_t[:], deg[:])
            s_t = sb.tile([P, GN, H], FP)
            nc.vector.tensor_reduce(
                s_t[:],
                psn_t[:].rearrange("p q t h -> p q h t"),
                mybir.AxisListType.X,
                mybir.AluOpType.add,
            )
            nc.vector.tensor_tensor(
                s_t[:], s_t[:],
                deg_t[:].unsqueeze(2).to_broadcast([P, GN, H]),
                mybir.AluOpType.add)
            r_t = sb.tile([P, GN, H], FP)
            nc.vector.reciprocal(r_t[:], s_t[:])
            nc.sync.dma_start(
                r_out[:].rearrange("(p q) h -> p q h", p=P), r_t[:])

    nc.compile()
    return nc


# ---------------------------------------------------------------------------
# Launch D: rescale
# ---------------------------------------------------------------------------

def _build_launch_d(batches, Gv, ktot):
    nc = bacc.Bacc("TRN2", target_bir_lowering=False, debug=False,
                   num_devices=NCORES)
    y_in = nc.dram_tensor("y_in", [P, ktot * H], F16, kind="ExternalInput")
    r_col = nc.dram_tensor("r_col", [P, ktot * H], F16, kind="ExternalInput")
    out = nc.dram_tensor("out", [P, ktot * H], F16, kind="ExternalOutput")

    with tile.TileContext(nc) as tc:
        with (
            tc.tile_pool(name="yp", bufs=3) as yp,
            tc.tile_pool(name="rp", bufs=3) as rp,
            tc.tile_pool(name="op", bufs=3) as op,
        ):
            for (g0, ng, dd, col0) in batches:
                cols = ng * dd
                y_t = yp.tile([P, cols, H], F16, tag="y")
                nc.sync.dma_start(
                    y_t[:],
                    y_in[:, col0 * H:(col0 + cols) * H]
                    .rearrange("p (c h) -> p c h", h=H))
                rcol_t = rp.tile([P, cols, H], F16, tag="rcol")
                nc.sync.dma_start(
                    rcol_t[:],
                    r_col[:, col0 * H:(col0 + cols) * H]
                    .rearrange("p (c h) -> p c h", h=H))
                o_t = op.tile([P, cols, H], F16, tag="o")
                # out = (y + 1) * r
                nc.vector.scalar_tensor_tensor(
                    o_t[:], y_t[:], 1.0, rcol_t[:],
                    mybir.AluOpType.add,
                    mybir.AluOpType.mult,
                )
                nc.sync.dma_start(
                    out[:, col0 * H:(col0 + cols) * H]
                    .rearrange("p (c h) -> p c h", h=H), o_t[:])

    nc.compile()
    return nc


# ---------------------------------------------------------------------------
# Host orchestration
# ---------------------------------------------------------------------------

def kernel(feat, etype, src, dst, W_fc, edge_emb, W_e, attn_l, attn_r, attn_e):
    feat = np.asarray(feat)
    etype = np.asarray(etype).astype(np.int64)
    src = np.asarray(src).astype(np.int64)
    dst = np.asarray(dst).astype(np.int64)
    W_fc = np.asarray(W_fc)
    edge_emb = np.asarray(edge_emb)
    W_e = np.asarray(W_e)
    attn_l = np.asarray(attn_l)
    attn_r = np.asarray(attn_r)
    attn_e = np.asarray(attn_e)

    # ---------------- Launch A ----------------
    nc_a = _build_launch_a()
    attn_lr = np.concatenate(
        [attn_l.reshape(1, H * O), attn_r.reshape(1, H * O)], axis=1)
    import ml_dtypes
    in_maps_a = []
    for s in range(NCORES):
        featT_s = np.zeros((IN, NSP), ml_dtypes.bfloat16)
        featT_s[:, :NS] = feat[s * NS:(s + 1) * NS].T.astype(ml_dtypes.bfloat16)
        in_maps_a.append({
            "featT": featT_s,
            "w_fc": W_fc.astype(np.float32),
            "attn_lr": np.broadcast_to(attn_lr.astype(np.float32), (P, 2 * H * O)).copy(),
            "edge_embT": np.ascontiguousarray(edge_emb.T.astype(np.float32)),
            "w_e": W_e.astype(np.float32),
            "attn_e": np.broadcast_to(attn_e.reshape(1, H * F).astype(np.float32), (T, H * F)).copy(),
        })
    res_a = run_bass_kernel_spmd(nc_a, in_maps_a, core_ids=list(range(NCORES)))

    el_full = np.concatenate(
        [res_a.results[s]["elp"][:NS] for s in range(NCORES)])   # [N, H]
    er_full = np.concatenate(
        [res_a.results[s]["erp"][:NS] for s in range(NCORES)])   # [N, H]
    eep = res_a.results[0]["eep"]                                # [T, H]
    table = np.ascontiguousarray(el_full.reshape(NBLK, 64))

    # ---------------- host index construction (integers only) -------------
    key = dst * T + etype
    order = np.argsort(key, kind="stable")          # by dst, then etype
    dst_sorted = dst[order]
    core_bounds = np.searchsorted(dst_sorted, [c * NS for c in range(NCORES + 1)])

    # per-core vr lists (vr = (local dst, etype) with cnt >= 1), count-sorted
    pc = []
    for c in range(NCORES):
        lo, hi = core_bounds[c], core_bounds[c + 1]
        e_ids = order[lo:hi]
        keys_c = key[order[lo:hi]] - c * NS * T     # local n*T + t
        vr_keys, vr_start, vr_cnt = np.unique(
            keys_c, return_index=True, return_counts=True)
        perm = np.argsort(vr_cnt, kind="stable")    # ascending cnt
        inv = np.empty(len(perm), np.int64)
        inv[perm] = np.arange(len(perm))
        pc.append(dict(e_ids=e_ids, keys_c=keys_c, vr_keys=vr_keys,
                       vr_start=vr_start, vr_cnt=vr_cnt, perm=perm, inv=inv))

    NV = max(len(p["vr_keys"]) for p in pc)
    Gv = (NV + P - 1) // P

    # shared per-group dd (max over cores)
    dd_g = np.zeros(Gv, np.int64)
    for c in range(NCORES):
        cnt_sorted = pc[c]["vr_cnt"][pc[c]["perm"]]
        padded = np.zeros(Gv * P, np.int64)
        padded[:len(cnt_sorted)] = cnt_sorted
        dd_g = np.maximum(dd_g, padded.reshape(Gv, P).max(axis=1))

    # batches of consecutive groups padded to a uniform dd (ascending dd)
    batches = []            # (g0, n_g, dd, col0)
    colstart_g = np.zeros(Gv, np.int64)
    col0 = 0
    g = 0
    while g < Gv:
        if dd_g[g] == 0:
            g += 1
            continue
        g0 = g
        ng = 1
        ddb = int(dd_g[g])
        while (g0 + ng < Gv and dd_g[g0 + ng] > 0
               and (ng + 1) * max(ddb, int(dd_g[g0 + ng])) <= COLS_MAX):
            ddb = max(ddb, int(dd_g[g0 + ng]))
            ng += 1
        for j in range(ng):
            colstart_g[g0 + j] = col0 + j * ddb
        batches.append((g0, ng, ddb, col0))
        col0 += ng * ddb
        g = g0 + ng
    ktot = col0

    nc_b = _build_launch_b(batches, Gv, ktot)

    dd_of_g = np.zeros(Gv, np.int64)
    for (g0, ng, ddb, c0) in batches:
        dd_of_g[g0:g0 + ng] = ddb
    assert (dd_g <= dd_of_g).all(), "batch width below group degree"

    # per-core B inputs
    in_maps_b = []
    slot_edge = np.full((NCORES, P, ktot), -1, np.int64)
    vr_pos = []             # per core: (node_local, t, p, g) per vr
    for c in range(NCORES):
        d = pc[c]
        nvc = len(d["vr_keys"])
        sortpos = d["inv"]                     # vr i -> sorted position
        g_of_vr = sortpos // P
        p_of_vr = sortpos % P

        # per-edge slot assignment
        vi = np.repeat(np.arange(nvc), d["vr_cnt"])
        rank = np.arange(len(d["e_ids"])) - d["vr_start"][vi]
        pp_ = p_of_vr[vi]
        cols_ = colstart_g[g_of_vr[vi]] + rank
        srcs = src[d["e_ids"]]

        idx_grid = np.zeros((P, ktot), np.int16)
        idx_grid[pp_, cols_] = (srcs >> 3).astype(np.int16)
        kk_grid = np.full((P, ktot), -1.0, np.float32)
        kk_grid[pp_, cols_] = (srcs & 7).astype(np.float32)
        slot_edge[c, pp_, cols_] = d["e_ids"]

        # wrapped idx: per batch, position i=(j*128+p) at [i%16, i//16], x8
        idx_all = np.zeros((P, 8 * ktot), np.int16)
        for (g0, ng, ddb, c0) in batches:
            cols = ng * ddb
            flat = idx_grid[:, c0:c0 + cols].T.reshape(-1)       # i = j*128+p
            w = flat.reshape(-1, 16).T                           # [16, ni/16]
            idx_all[:, 8 * c0:8 * (c0 + cols)] = np.tile(w, (8, 1))

        # per-vr er'/ee' grids
        nodes_l = d["vr_keys"] // T
        ts = d["vr_keys"] % T
        er_g = np.zeros((P, Gv, H), np.float32)
        ee_g = np.zeros((P, Gv, H), np.float32)
        er_g[p_of_vr, g_of_vr] = er_full[c * NS + nodes_l]
        ee_g[p_of_vr, g_of_vr] = eep[ts]
        vr_pos.append((nodes_l, ts, p_of_vr, g_of_vr))

        in_maps_b.append({
            "table": table,
            "idx_all": idx_all,
            "kk_all": kk_grid.astype(np.float16),
            "iota": np.broadcast_to(
                np.arange(8, dtype=np.float16), (P, 8)).copy(),
            "er_vr": er_g.reshape(P, Gv * H).astype(np.float16),
            "ee_vr": ee_g.reshape(P, Gv * H).astype(np.float16),
        })

    res_b = run_bass_kernel_spmd(nc_b, in_maps_b, core_ids=list(range(NCORES)))

    # ---------------- Launch C ----------------
    nc_c = _build_launch_c()
    degs = np.bincount(dst, minlength=N)
    in_maps_c = []
    for c in range(NCORES):
        ps_c = res_b.results[c]["ps_out"].reshape(P, Gv, H)
        nodes_l, ts, p_v, g_v = vr_pos[c]
        psn = np.zeros((NSP, T, H), np.float32)
        psn[nodes_l, ts] = ps_c[p_v, g_v]
        deg_c = np.zeros(NSP, np.float32)
        deg_c[:NS] = degs[c * NS:(c + 1) * NS]
        deg_c = np.maximum(deg_c, 1.0)
        in_maps_c.append({
            "psn": psn.reshape(NSP, T * H).astype(np.float16),
            "deg": deg_c.reshape(P, GN),
        })
    res_c = run_bass_kernel_spmd(nc_c, in_maps_c, core_ids=list(range(NCORES)))

    # ---------------- Launch D ----------------
    nc_d = _build_launch_d(batches, Gv, ktot)
    in_maps_d = []
    for c in range(NCORES):
        r_c = res_c.results[c]["r_out"]        # [NSP, H] node-major
        nodes_l, ts, p_v, g_v = vr_pos[c]
        r_g = np.zeros((P, Gv, H), np.float32)
        r_g[p_v, g_v] = r_c[nodes_l]
        # expand per-vr r to per-column (replication by integer indexing)
        r_col = np.empty((P, ktot, H), np.float32)
        for (g0, ng, ddb, c0) in batches:
            r_col[:, c0:c0 + ng * ddb] = np.repeat(
                r_g[:, g0:g0 + ng], ddb, axis=1)
        in_maps_d.append({
            "y_in": res_b.results[c]["y_out"],
            "r_col": r_col.reshape(P, ktot * H).astype(np.float16),
        })
    res_d = run_bass_kernel_spmd(nc_d, in_maps_d, core_ids=list(range(NCORES)))

    # ---------------- unshard ----------------
    out = np.zeros((E, H), np.float32)
    for c in range(NCORES):
        o_c = res_d.results[c]["out"].reshape(P, ktot, H).astype(np.float32)
        mask = slot_edge[c] >= 0
        out[slot_edge[c][mask]] = o_c[mask]

    # timing estimate via the cost-model simulator
    try:
        from concourse.timeline_sim import TimelineSim
        _timings["A_ns"] = TimelineSim(nc_a).simulate()
        _timings["B_ns"] = TimelineSim(nc_b).simulate()
        _timings["C_ns"] = TimelineSim(nc_c).simulate()
        _timings["D_ns"] = TimelineSim(nc_d).simulate()
    except Exception as ex:  # timing must never break correctness
        _timings["error"] = repr(ex)

    return out
